# revision 19
# baseline (speedup 1.0000x reference)
"""Trainium2 Bass kernel for nn_DigitCaps (capsule dynamic routing with
piecewise-linear squash). Self-contained: hardcodes shapes/sharding.

Sharding: data-parallel over batch (100 -> 8 cores x 13, zero-padded to 104).
W is replicated. The per-routing-iteration mean over batch is an AllReduce of
per-core partial sums of a_ij.

Device algorithm per core (batch chunk Bc=13):
  W stays resident in SBUF as w[(rr,i)=128, (blk=72, c=10, o=16)].
  s_j^t      = sum_{r,i} (c^t[r,c] * W[r,c,o,i]) * x0[b,r,i]   (PE, fp32)
  squash     = rank-based piecewise-linear remap (DVE, no actual sort needed)
  partial_a  = sum_{o,i} W * G,  G[(r,i),(c,o)] = sum_b x0[b,r,i] v[b,c,o]
               (PE for G and the i-reduction, DVE for the o-reduction)
  b_ij      += AllReduce(partial_a) / 100
"""

import numpy as np

B, R, C, O, II = 100, 1152, 10, 16, 8
NCORES = 8
BC = 13          # per-core batch (zero-padded to 8*13 = 104)
BLK = R // 16    # 72 blocks of 16 routes
CO = C * O       # 160

T1, T2, T3 = -0.075410217, 0.0, 0.062207676
SEGS = [(-0.074520095, 0.349297946), (-0.534473989, 0.27196494),
        (0.637642944, 0.295330779), (0.169344703, 0.353784456)]

_RUNNER = [None]
_NC = [None]
_IO = [None]
_SHARDED = [None]


def _build_nc():
    import concourse.bacc as bacc
    import concourse.mybir as mybir
    from concourse import tile

    dt = mybir.dt
    f32 = dt.float32
    Alu = mybir.AluOpType
    Act = mybir.ActivationFunctionType

    nc = bacc.Bacc("TRN2", target_bir_lowering=False, debug=False,
                   num_devices=NCORES)

    w_in = nc.dram_tensor("w_t", [128, BLK * CO], f32, kind="ExternalInput")
    xt_in = nc.dram_tensor("x_t", [128, BLK * BC], f32, kind="ExternalInput")
    xb2_in = nc.dram_tensor("x_b2", [64, (BLK // 2) * 128], f32,
                            kind="ExternalInput")
    eye16_in = nc.dram_tensor("eye16", [128, 16], f32, kind="ExternalInput")
    e16x128_in = nc.dram_tensor("e16x128", [16, 128], f32, kind="ExternalInput")
    ones16_in = nc.dram_tensor("ones16", [16, 1], f32, kind="ExternalInput")
    ones1x16_in = nc.dram_tensor("ones1x16", [1, 16], f32, kind="ExternalInput")
    ident_in = nc.dram_tensor("ident", [128, 128], f32, kind="ExternalInput")

    v_out = nc.dram_tensor("v_out", [BC, CO], f32, kind="ExternalOutput")
    smut_out = nc.dram_tensor("smut_out", [BC, CO], f32, kind="ExternalOutput")
    c_out = nc.dram_tensor("c_out", [16, BLK * C], f32, kind="ExternalOutput")

    with tile.TileContext(nc) as tc:
        with tc.tile_pool(name="res", bufs=1) as res, \
             tc.tile_pool(name="work", bufs=2) as work, \
             tc.tile_pool(name="ps", bufs=1, space="PSUM") as ps, \
             tc.tile_pool(name="dram", bufs=1, space="DRAM") as dram:

            # ---- resident tensors ----
            w_sb = res.tile([128, BLK * CO], f32, tag="w_sb")
            wc_sb = res.tile([128, BLK * CO], f32, tag="wc_sb")
            xt_sb = res.tile([128, BLK * BC], f32, tag="xt_sb")
            xb2_sb = res.tile([64, (BLK // 2) * 128], f32, tag="xb2_sb")
            xb2_r = res.tile([64, (BLK // 2) * 128], dt.float32r,
                             tag="xb2_r")
            eye16 = res.tile([128, 16], f32, tag="eye16")
            e16x128 = res.tile([16, 128], f32, tag="e16x128")
            ones16 = res.tile([16, 1], f32, tag="ones16")
            ones1x16 = res.tile([1, 16], f32, tag="ones1x16")
            ident = res.tile([128, 128], f32, tag="ident")
            b16 = res.tile([16, BLK * C], f32, tag="b16")
            t2 = res.tile([128, BLK * C], f32, tag="t2")

            nc.sync.dma_start(xt_sb[:], xt_in[:])
            nc.sync.dma_start(xb2_sb[:], xb2_in[:])
            WCH = BLK * CO // 8
            _dma_engs = [nc.sync, nc.scalar, nc.sync, nc.scalar,
                         nc.sync, nc.scalar, nc.gpsimd, nc.gpsimd]
            for ch in range(8):
                _dma_engs[ch].dma_start(
                    w_sb[:, ch * WCH:(ch + 1) * WCH],
                    w_in[:, ch * WCH:(ch + 1) * WCH])
            nc.sync.dma_start(eye16[:], eye16_in[:])
            nc.sync.dma_start(e16x128[:], e16x128_in[:])
            nc.sync.dma_start(ones16[:], ones16_in[:])
            nc.sync.dma_start(ones1x16[:], ones1x16_in[:])
            nc.sync.dma_start(ident[:], ident_in[:])

            w4 = w_sb[:].rearrange("p (blk c o) -> p blk c o", blk=BLK, c=C)
            wc4 = wc_sb[:].rearrange("p (blk c o) -> p blk c o", blk=BLK, c=C)
            xt3 = xt_sb[:].rearrange("p (blk b) -> p blk b", blk=BLK)

            # squash scratch (shared across iterations via tags)
            sT = res.tile([BC, CO], f32, tag="sT")        # s^T, then s_mut^T
            vT = res.tile([BC, CO], f32, tag="vT")
            vTd = res.tile([64, 512], dt.float32r, tag="vTd")
            nc.gpsimd.tensor_copy(xb2_r[:], xb2_sb[:])
            nc.vector.tensor_single_scalar(vTd[:], xb2_sb[:, 0:512], 0.0,
                                           Alu.mult)
            lt = res.tile([BC, C * C], f32, tag="lt")
            rank = res.tile([BC, C], f32, tag="rank")
            nf = res.tile([BC, C], f32, tag="nf")
            cnt = res.tile([BC, 12], f32, tag="cnt")      # i1,i2,i3,i1m,...,gates
            segs = res.tile([BC, 4 * C], f32, tag="segs")
            msk = res.tile([BC, 4 * C], f32, tag="msk")
            mski = res.tile([BC, 4 * C], dt.int32, tag="mski")

            exp16 = res.tile([16, BLK * C], f32, tag="exp16")
            cij16 = res.tile([16, BLK * C], f32, tag="cij16")
            rz = res.tile([1, C], f32, tag="rz")
            rcp16 = res.tile([16, C], f32, tag="rcp16")
            ar_sb = res.tile([16, BLK * C], f32, tag="ar_sb")
            c_sb = res.tile([128, BLK * C], f32, tag="c_sb")

            def s_pass(lhs4, scale):
                """s^T [BC, CO] <- (sum over blocks of lhsT.T @ x) transposed."""
                s_psA = ps.tile([128, BC], f32, tag="pA")
                s_psB = ps.tile([32, BC], f32, tag="pB")
                for blk in range(BLK):
                    nc.tensor.matmul(s_psA[:], lhs4[:, blk, 0:8, :],
                                     xt3[:, blk, :],
                                     start=(blk == 0), stop=(blk == BLK - 1))
                for blk in range(BLK):
                    nc.tensor.matmul(s_psB[:], lhs4[:, blk, 8:10, :],
                                     xt3[:, blk, :],
                                     start=(blk == 0), stop=(blk == BLK - 1))
                sA = work.tile([128, BC], f32, tag="sA")
                sB = work.tile([32, BC], f32, tag="sB")
                nc.scalar.mul(sA[:], s_psA[:], scale)
                nc.scalar.mul(sB[:], s_psB[:], scale)
                tTA = ps.tile([BC, 128], f32, tag="pC")
                tTB = ps.tile([BC, 32], f32, tag="pD")
                nc.tensor.transpose(tTA[:], sA[:], ident[:])
                nc.tensor.transpose(tTB[:], sB[:], ident[:32, :32])
                nc.vector.tensor_copy(sT[:, 0:128], tTA[:])
                nc.vector.tensor_copy(sT[:, 128:160], tTB[:])

            def squash():
                """sT -> (s_mut^T in sT, v^T in vT, new_f in nf)."""
                fv = sT[:, 0:CO:16]                       # [BC, 10] channel-0
                lt3 = lt[:].rearrange("p (a b) -> p a b", a=C)
                nc.vector.tensor_tensor(
                    lt3, fv.unsqueeze(1).broadcast_to([BC, C, C]),
                    fv.unsqueeze(2).broadcast_to([BC, C, C]), Alu.is_lt)
                nc.vector.reduce_sum(rank[:], lt3, axis=mybir.AxisListType.X)
                # counts: i_k = #(f < T_k); then i_k - 1; then gates
                i1, i2, i3 = cnt[:, 0:1], cnt[:, 1:2], cnt[:, 2:3]
                i1m, i2m, i3m = cnt[:, 3:4], cnt[:, 4:5], cnt[:, 5:6]
                g1, g2, g3, g4 = (cnt[:, 6:7], cnt[:, 7:8],
                                  cnt[:, 8:9], cnt[:, 9:10])
                tmp = cnt[:, 10:11]
                for thr, acc in ((T1, i1), (T2, i2), (T3, i3)):
                    nc.vector.tensor_single_scalar(
                        lt[:, 0:C], fv, float(thr), Alu.is_lt)
                    nc.vector.reduce_sum(acc, lt[:, 0:C].unsqueeze(1),
                                         axis=mybir.AxisListType.X)
                nc.vector.tensor_scalar_add(i1m, i1, -1.0)
                nc.vector.tensor_scalar_add(i2m, i2, -1.0)
                nc.vector.tensor_scalar_add(i3m, i3, -1.0)
                nc.vector.tensor_single_scalar(g1, i1, 0.0, Alu.is_gt)
                nc.vector.tensor_single_scalar(tmp, i2, 0.0, Alu.is_gt)
                nc.vector.scalar_tensor_tensor(g2, i2, i1, tmp,
                                               Alu.is_gt, Alu.mult)
                nc.vector.tensor_single_scalar(tmp, i3, 0.0, Alu.is_gt)
                nc.vector.scalar_tensor_tensor(g3, i3, i2, tmp,
                                               Alu.is_gt, Alu.mult)
                nc.vector.tensor_single_scalar(g4, i3, float(C), Alu.is_lt)
                # segments seg_k = a_k * f + b_k
                for k, (a, b) in enumerate(SEGS):
                    nc.vector.tensor_scalar(segs[:, k * C:(k + 1) * C], fv,
                                            float(a), float(b),
                                            Alu.mult, Alu.add)
                # masks (disjoint)
                m1, m2 = msk[:, 0:C], msk[:, C:2 * C]
                m3, m4 = msk[:, 2 * C:3 * C], msk[:, 3 * C:4 * C]
                nc.vector.scalar_tensor_tensor(
                    m1, rank[:], i1m, g1.broadcast_to([BC, C]),
                    Alu.is_lt, Alu.mult)
                nc.vector.scalar_tensor_tensor(
                    m2, rank[:], i1, g2.broadcast_to([BC, C]),
                    Alu.is_ge, Alu.mult)
                nc.vector.scalar_tensor_tensor(
                    m2, rank[:], i2m, m2, Alu.is_lt, Alu.mult)
                nc.vector.scalar_tensor_tensor(
                    m3, rank[:], i2, g3.broadcast_to([BC, C]),
                    Alu.is_ge, Alu.mult)
                nc.vector.scalar_tensor_tensor(
                    m3, rank[:], i3m, m3, Alu.is_lt, Alu.mult)
                nc.vector.scalar_tensor_tensor(
                    m4, rank[:], i3, g4.broadcast_to([BC, C]),
                    Alu.is_ge, Alu.mult)
                nc.vector.scalar_tensor_tensor(
                    m4, rank[:], float(C - 1), m4, Alu.is_lt, Alu.mult)
                # new_f = select chain (masks disjoint, any order)
                nc.vector.tensor_copy(mski[:], msk[:])
                nc.vector.tensor_copy(nf[:], fv)
                for k in range(4):
                    nc.vector.copy_predicated(nf[:], mski[:, k * C:(k + 1) * C],
                                              segs[:, k * C:(k + 1) * C])
                # s_mut channel 0 <- new_f ; v = new_f * s_mut
                nc.vector.tensor_copy(fv, nf[:])
                nfb = nf[:].unsqueeze(2).broadcast_to([BC, C, O])
                nc.vector.tensor_tensor(
                    vT[:].rearrange("p (c o) -> p c o", c=C), sT[:].rearrange(
                        "p (c o) -> p c o", c=C), nfb, Alu.mult)
                nc.vector.tensor_copy(vTd[0:BC, 0:CO], vT[:])
                nc.vector.tensor_copy(vTd[32:32 + BC, 256:256 + CO], vT[:])

            def a_pass(cc_in, cc_out):
                """partial_a -> AllReduce -> ar_sb [16, (blk c)]."""
                GRP = 4
                for g0 in range(0, BLK, GRP):
                    g = g0 // 2
                    g_ps = ps.tile([128, 1024], f32,
                                   tag=("pA" if (g0 // GRP) % 2 == 0 else "pB"))
                    nc.tensor.matmul(
                        g_ps[:, 0:512], xb2_r[:, g * 128:(g + 1) * 128],
                        vTd[:], start=True, stop=True)
                    nc.tensor.matmul(
                        g_ps[:, 512:1024], xb2_r[:, (g + 1) * 128:(g + 2) * 128],
                        vTd[:], start=True, stop=True)
                    prod = work.tile([128, GRP * CO], f32, tag="prod")
                    g_view = g_ps[:].rearrange("p (g n) -> p g n", g=GRP)[
                        :, :, 0:CO]
                    w_view = w_sb[:, g0 * CO:(g0 + GRP) * CO].rearrange(
                        "p (g n) -> p g n", g=GRP)
                    if (g0 // GRP) % 3:
                        nc.vector.tensor_tensor(
                            prod[:].rearrange("p (g n) -> p g n", g=GRP),
                            w_view, g_view, Alu.mult)
                        nc.vector.reduce_sum(
                            t2[:, g0 * C:(g0 + GRP) * C].rearrange(
                                "p (g c) -> p g c", g=GRP),
                            prod[:].rearrange("p (g c o) -> p g c o",
                                              g=GRP, c=C),
                            axis=mybir.AxisListType.X)
                    else:
                        g_sb = work.tile([128, GRP * CO], f32, tag="g_sb")
                        nc.scalar.copy(
                            g_sb[:].rearrange("p (g n) -> p g n", g=GRP),
                            g_view)
                        nc.gpsimd.tensor_tensor(
                            prod[:], w_sb[:, g0 * CO:(g0 + GRP) * CO],
                            g_sb[:], Alu.mult)
                        p4 = prod[:].rearrange("p (g c o) -> p g c o",
                                               g=GRP, c=C)
                        nc.gpsimd.tensor_add(p4[:, :, :, 0:8], p4[:, :, :, 0:8],
                                             p4[:, :, :, 8:16])
                        nc.gpsimd.tensor_add(p4[:, :, :, 0:4], p4[:, :, :, 0:4],
                                             p4[:, :, :, 4:8])
                        nc.gpsimd.tensor_add(p4[:, :, :, 0:2], p4[:, :, :, 0:2],
                                             p4[:, :, :, 2:4])
                        nc.gpsimd.tensor_add(
                            t2[:, g0 * C:(g0 + GRP) * C].rearrange(
                                "p (g c) -> p g c", g=GRP).unsqueeze(3),
                            p4[:, :, :, 0:1], p4[:, :, :, 1:2])
                a16 = ps.tile([16, BLK * C], f32, tag="pC")
                nc.tensor.matmul(a16[:, 0:512], eye16[:], t2[:, 0:512],
                                 start=True, stop=True)
                nc.tensor.matmul(a16[:, 512:BLK * C], eye16[:],
                                 t2[:, 512:BLK * C], start=True, stop=True)
                a16_sb = work.tile([16, BLK * C], f32, tag="a16_sb")
                nc.scalar.copy(a16_sb[:], a16[:])
                nc.sync.dma_start(cc_in[:], a16_sb[:])
                nc.gpsimd.collective_compute(
                    "AllReduce", Alu.add,
                    ins=[cc_in.opt()], outs=[cc_out.opt()],
                    replica_groups=[list(range(NCORES))])
                nc.sync.dma_start(ar_sb[:], cc_out[:])

            def softmax_and_wc():
                """cij16 <- softmax(b16 over r); wc <- W * c (bcast over o)."""
                nc.scalar.activation(exp16[:], b16[:], Act.Exp)
                zr16 = res.tile([16, C], f32, tag="zr16")
                nc.vector.reduce_sum(
                    zr16[:],
                    exp16[:].rearrange("p (blk c) -> p c blk", blk=BLK),
                    axis=mybir.AxisListType.X)
                z_ps = ps.tile([1, C], f32, tag="pD")
                nc.tensor.matmul(z_ps[:], ones16[:], zr16[:],
                                 start=True, stop=True)
                nc.vector.reciprocal(rz[:], z_ps[:])
                r_ps = ps.tile([16, C], f32, tag="pE")
                nc.tensor.matmul(r_ps[:], ones1x16[:], rz[:],
                                 start=True, stop=True)
                nc.vector.tensor_copy(rcp16[:], r_ps[:])
                nc.vector.tensor_tensor(
                    cij16[:].rearrange("p (blk c) -> p blk c", blk=BLK),
                    exp16[:].rearrange("p (blk c) -> p blk c", blk=BLK),
                    rcp16[:].unsqueeze(1).broadcast_to([16, BLK, C]),
                    Alu.mult)
                c_ps = ps.tile([128, BLK * C], f32, tag="pC")
                nc.tensor.matmul(c_ps[:, 0:512], e16x128[:], cij16[:, 0:512],
                                 start=True, stop=True)
                nc.tensor.matmul(c_ps[:, 512:BLK * C], e16x128[:],
                                 cij16[:, 512:BLK * C], start=True, stop=True)
                nc.scalar.copy(c_sb[:], c_ps[:])
                cp3 = c_ps[:].rearrange("p (blk c) -> p blk c", blk=BLK)
                cs3 = c_sb[:].rearrange("p (blk c) -> p blk c", blk=BLK)
                for k, ch in enumerate(range(0, BLK, 6)):
                    if k % 3 == 2:
                        nc.gpsimd.tensor_tensor(
                            wc4[:, ch:ch + 6], w4[:, ch:ch + 6],
                            cs3[:, ch:ch + 6].unsqueeze(3)
                            .broadcast_to([128, 6, C, O]), Alu.mult)
                    else:
                        nc.vector.tensor_tensor(
                            wc4[:, ch:ch + 6], w4[:, ch:ch + 6],
                            cp3[:, ch:ch + 6].unsqueeze(3)
                            .broadcast_to([128, 6, C, O]), Alu.mult)

            cc_in0 = dram.tile([16, BLK * C], f32, tag="cc_in0")
            cc_out0 = dram.tile([16, BLK * C], f32, tag="cc_out0")
            cc_in1 = dram.tile([16, BLK * C], f32, tag="cc_in1")
            cc_out1 = dram.tile([16, BLK * C], f32, tag="cc_out1")

            # ---- iteration 0: c uniform = 1/R ----
            s_pass(w4, 1.0 / R)
            squash()
            a_pass(cc_in0, cc_out0)
            # b16 = ar/100
            nc.vector.tensor_scalar_mul(b16[:], ar_sb[:], 1.0 / B)

            # ---- iteration 1 ----
            softmax_and_wc()
            s_pass(wc4, 1.0)
            squash()
            a_pass(cc_in1, cc_out1)
            nc.vector.scalar_tensor_tensor(b16[:], ar_sb[:], 1.0 / B, b16[:],
                                           Alu.mult, Alu.add)

            # ---- iteration 2 (final) ----
            softmax_and_wc()
            nc.sync.dma_start(c_out[:], cij16[:])
            s_pass(wc4, 1.0)
            squash()
            nc.sync.dma_start(v_out[:], vT[:])
            nc.sync.dma_start(smut_out[:], sT[:])

    nc.finalize()
    return nc


def _make_runner():
    """Build nc once and a cached jitted SPMD executor (axon/PJRT path)."""
    import jax
    import jax.numpy as jnp
    from jax.sharding import Mesh, PartitionSpec
    from jax.experimental.shard_map import shard_map
    import concourse.mybir as mybir
    from concourse import bass2jax

    nc = _build_nc()
    bass2jax.install_neuronx_cc_hook()

    in_names, out_names, out_avals, zero_outs = [], [], [], []
    partition_name = (nc.partition_id_tensor.name
                      if nc.partition_id_tensor else None)
    for alloc in nc.m.functions[0].allocations:
        if not isinstance(alloc, mybir.MemoryLocationSet):
            continue
        name = alloc.memorylocations[0].name
        if alloc.kind == "ExternalInput":
            if name != partition_name:
                in_names.append(name)
        elif alloc.kind == "ExternalOutput":
            shape = tuple(alloc.tensor_shape)
            dtype = mybir.dt.np(alloc.dtype)
            out_names.append(name)
            out_avals.append(jax.core.ShapedArray(shape, dtype))
            zero_outs.append(np.zeros(shape, dtype))
    n_params = len(in_names)
    n_outs = len(out_avals)
    all_in_names = list(in_names) + list(out_names)
    if partition_name is not None:
        all_in_names.append(partition_name)
    donate = tuple(range(n_params, n_params + n_outs))

    def _body(*args):
        operands = list(args)
        if partition_name is not None:
            operands.append(bass2jax.partition_id_tensor())
        outs = bass2jax._bass_exec_p.bind(
            *operands,
            out_avals=tuple(out_avals),
            in_names=tuple(all_in_names),
            out_names=tuple(out_names),
            lowering_input_output_aliases=(),
            sim_require_finite=False,
            sim_require_nnan=False,
            nc=nc,
        )
        return tuple(outs)

    devices = jax.devices()[:NCORES]
    mesh = Mesh(np.asarray(devices), ("core",))
    in_specs = (PartitionSpec("core"),) * (n_params + n_outs)
    out_specs = (PartitionSpec("core"),) * n_outs
    sharded = jax.jit(
        shard_map(_body, mesh=mesh, in_specs=in_specs, out_specs=out_specs,
                  check_rep=False),
        donate_argnums=donate, keep_unused=True)
    _NC[0] = nc
    _IO[0] = (in_names, out_names, out_avals, zero_outs)
    _SHARDED[0] = sharded

    def run(in_maps):
        per_core = [[np.asarray(m[k]) for k in in_names] for m in in_maps]
        concat_in = [np.concatenate([per_core[c][i] for c in range(NCORES)],
                                    axis=0) for i in range(n_params)]
        concat_zeros = [np.zeros((NCORES * z.shape[0], *z.shape[1:]), z.dtype)
                        for z in zero_outs]
        out_arrs = sharded(*concat_in, *concat_zeros)
        outs = [np.asarray(o) for o in out_arrs]
        return [
            {name: outs[i].reshape(NCORES, *out_avals[i].shape)[c]
             for i, name in enumerate(out_names)}
            for c in range(NCORES)
        ]

    return run


def _prep_inputs(x0):
    """Host-side shard/reformat (slicing, transposes, padding only)."""
    x0p = np.zeros((NCORES * BC, R, II), np.float32)
    x0p[:B] = x0
    w = None  # filled by caller
    eye16 = np.zeros((128, 16), np.float32)
    eye16[np.arange(128), np.arange(128) // 8] = 1.0
    e16x128 = np.zeros((16, 128), np.float32)
    e16x128[np.arange(128) // 8, np.arange(128)] = 1.0
    ones16 = np.ones((16, 1), np.float32)
    ones1x16 = np.ones((1, 16), np.float32)
    ident = np.eye(128, dtype=np.float32)
    per_core = []
    for ci in range(NCORES):
        xc = x0p[ci * BC:(ci + 1) * BC]           # [13, 1152, 8]
        x_t = np.ascontiguousarray(
            xc.reshape(BC, BLK, 16, II).transpose(2, 3, 1, 0)
        ).reshape(128, BLK * BC)
        xpair = xc.reshape(BC, BLK // 2, 2, 128)
        x_b2 = np.zeros((64, (BLK // 2) * 128), np.float32)
        x_b2[0:BC] = np.ascontiguousarray(
            xpair[:, :, 0, :]).reshape(BC, -1)
        x_b2[32:32 + BC] = np.ascontiguousarray(
            xpair[:, :, 1, :]).reshape(BC, -1)
        per_core.append({"x_t": x_t, "x_b2": x_b2, "eye16": eye16,
                         "e16x128": e16x128, "ones16": ones16,
                         "ones1x16": ones1x16, "ident": ident})
    return per_core


def kernel(x0, x1, x2, W, train_or_test=0, epch=0):
    x0 = np.asarray(x0, np.float32)
    W = np.asarray(W, np.float32)
    if _RUNNER[0] is None:
        _RUNNER[0] = _make_runner()
    run = _RUNNER[0]

    w_t = np.ascontiguousarray(
        W[0].reshape(BLK, 16, C, O, II).transpose(1, 4, 0, 2, 3)
    ).reshape(128, BLK * CO)
    in_maps = _prep_inputs(x0)
    for m in in_maps:
        m["w_t"] = w_t

    results = run(in_maps)

    v = np.concatenate([r["v_out"] for r in results], axis=0)[:B]
    smut = np.concatenate([r["smut_out"] for r in results], axis=0)[:B]
    c16 = results[0]["c_out"].reshape(16, BLK, C)
    c2 = np.ascontiguousarray(c16.transpose(1, 0, 2)).reshape(R, C)

    v_j = v.reshape(B, C, O, 1)
    s_mut = smut.reshape(B, C, O, 1)
    c_full = np.broadcast_to(c2[None, :, :, None, None], (B, R, C, 1, 1))
    Wb = np.broadcast_to(W, (B,) + W.shape[1:])
    squashed_u = np.broadcast_to(x0[:, :, None, :, None], (B, R, C, II, 1))
    return (v_j, c_full, Wb, squashed_u, np.asarray(x1), np.asarray(x2),
            s_mut)


# revision 20
# speedup vs baseline: 1.0114x; 1.0114x over previous
"""Trainium2 Bass kernel for nn_DigitCaps (capsule dynamic routing with
piecewise-linear squash). Self-contained: hardcodes shapes/sharding.

Sharding: data-parallel over batch (100 -> 8 cores x 13, zero-padded to 104).
W is replicated. The per-routing-iteration mean over batch is an AllReduce of
per-core partial sums of a_ij.

Device algorithm per core (batch chunk Bc=13):
  W stays resident in SBUF as w[(rr,i)=128, (blk=72, c=10, o=16)].
  s_j^t      = sum_{r,i} (c^t[r,c] * W[r,c,o,i]) * x0[b,r,i]   (PE, fp32)
  squash     = rank-based piecewise-linear remap (DVE, no actual sort needed)
  partial_a  = sum_{o,i} W * G,  G[(r,i),(c,o)] = sum_b x0[b,r,i] v[b,c,o]
               (PE for G and the i-reduction, DVE for the o-reduction)
  b_ij      += AllReduce(partial_a) / 100
"""

import numpy as np

B, R, C, O, II = 100, 1152, 10, 16, 8
NCORES = 8
BC = 13          # per-core batch (zero-padded to 8*13 = 104)
BLK = R // 16    # 72 blocks of 16 routes
CO = C * O       # 160

T1, T2, T3 = -0.075410217, 0.0, 0.062207676
SEGS = [(-0.074520095, 0.349297946), (-0.534473989, 0.27196494),
        (0.637642944, 0.295330779), (0.169344703, 0.353784456)]

_RUNNER = [None]
_NC = [None]
_IO = [None]
_SHARDED = [None]


def _build_nc():
    import concourse.bacc as bacc
    import concourse.mybir as mybir
    from concourse import tile

    dt = mybir.dt
    f32 = dt.float32
    Alu = mybir.AluOpType
    Act = mybir.ActivationFunctionType

    nc = bacc.Bacc("TRN2", target_bir_lowering=False, debug=False,
                   num_devices=NCORES)

    w_in = nc.dram_tensor("w_t", [128, BLK * CO], f32, kind="ExternalInput")
    xt_in = nc.dram_tensor("x_t", [128, BLK * BC], f32, kind="ExternalInput")
    xb2_in = nc.dram_tensor("x_b2", [64, (BLK // 2) * 128], f32,
                            kind="ExternalInput")
    eye16_in = nc.dram_tensor("eye16", [128, 16], f32, kind="ExternalInput")
    e16x128_in = nc.dram_tensor("e16x128", [16, 128], f32, kind="ExternalInput")
    ones16_in = nc.dram_tensor("ones16", [16, 1], f32, kind="ExternalInput")
    ones1x16_in = nc.dram_tensor("ones1x16", [1, 16], f32, kind="ExternalInput")
    ident_in = nc.dram_tensor("ident", [128, 128], f32, kind="ExternalInput")

    v_out = nc.dram_tensor("v_out", [BC, CO], f32, kind="ExternalOutput")
    smut_out = nc.dram_tensor("smut_out", [BC, CO], f32, kind="ExternalOutput")
    c_out = nc.dram_tensor("c_out", [16, BLK * C], f32, kind="ExternalOutput")

    with tile.TileContext(nc) as tc:
        with tc.tile_pool(name="res", bufs=1) as res, \
             tc.tile_pool(name="work", bufs=2) as work, \
             tc.tile_pool(name="ps", bufs=1, space="PSUM") as ps, \
             tc.tile_pool(name="dram", bufs=1, space="DRAM") as dram:

            # ---- resident tensors ----
            w_sb = res.tile([128, BLK * CO], f32, tag="w_sb")
            wc_sb = res.tile([128, BLK * CO], f32, tag="wc_sb")
            xt_sb = res.tile([128, BLK * BC], f32, tag="xt_sb")
            xb2_sb = res.tile([64, (BLK // 2) * 128], f32, tag="xb2_sb")
            xb2_r = res.tile([64, (BLK // 2) * 128], dt.float32r,
                             tag="xb2_r")
            eye16 = res.tile([128, 16], f32, tag="eye16")
            e16x128 = res.tile([16, 128], f32, tag="e16x128")
            ones16 = res.tile([16, 1], f32, tag="ones16")
            ones1x16 = res.tile([1, 16], f32, tag="ones1x16")
            ident = res.tile([128, 128], f32, tag="ident")
            b16 = res.tile([16, BLK * C], f32, tag="b16")
            t2 = res.tile([128, BLK * C], f32, tag="t2")

            nc.sync.dma_start(xt_sb[:], xt_in[:])
            nc.sync.dma_start(xb2_sb[:], xb2_in[:])
            WCH = BLK * CO // 8
            _dma_engs = [nc.sync, nc.scalar, nc.sync, nc.scalar,
                         nc.sync, nc.scalar, nc.gpsimd, nc.gpsimd]
            for ch in range(8):
                _dma_engs[ch].dma_start(
                    w_sb[:, ch * WCH:(ch + 1) * WCH],
                    w_in[:, ch * WCH:(ch + 1) * WCH])
            nc.sync.dma_start(eye16[:], eye16_in[:])
            nc.sync.dma_start(e16x128[:], e16x128_in[:])
            nc.sync.dma_start(ones16[:], ones16_in[:])
            nc.sync.dma_start(ones1x16[:], ones1x16_in[:])
            nc.sync.dma_start(ident[:], ident_in[:])

            w4 = w_sb[:].rearrange("p (blk c o) -> p blk c o", blk=BLK, c=C)
            wc4 = wc_sb[:].rearrange("p (blk c o) -> p blk c o", blk=BLK, c=C)
            xt3 = xt_sb[:].rearrange("p (blk b) -> p blk b", blk=BLK)

            # squash scratch (shared across iterations via tags)
            sT = res.tile([BC, CO], f32, tag="sT")        # s^T, then s_mut^T
            vT = res.tile([BC, CO], f32, tag="vT")
            vTd = res.tile([64, 512], dt.float32r, tag="vTd")
            nc.gpsimd.tensor_copy(xb2_r[:], xb2_sb[:])
            nc.vector.tensor_single_scalar(vTd[:], xb2_sb[:, 0:512], 0.0,
                                           Alu.mult)
            lt = res.tile([BC, C * C], f32, tag="lt")
            rank = res.tile([BC, C], f32, tag="rank")
            nf = res.tile([BC, C], f32, tag="nf")
            cnt = res.tile([BC, 12], f32, tag="cnt")      # i1,i2,i3,i1m,...,gates
            segs = res.tile([BC, 4 * C], f32, tag="segs")
            msk = res.tile([BC, 4 * C], f32, tag="msk")
            mski = res.tile([BC, 4 * C], dt.int32, tag="mski")

            exp16 = res.tile([16, BLK * C], f32, tag="exp16")
            cij16 = res.tile([16, BLK * C], f32, tag="cij16")
            rz = res.tile([1, C], f32, tag="rz")
            rcp16 = res.tile([16, C], f32, tag="rcp16")
            rcp13 = res.tile([BC, C], f32, tag="rcp13")
            ar_sb = res.tile([16, BLK * C], f32, tag="ar_sb")
            c_sb = res.tile([128, BLK * C], f32, tag="c_sb")

            def s_pass(lhs4, scale, norm=None):
                """s^T [BC, CO] <- (sum over blocks of lhsT.T @ x) transposed.
                norm: optional [BC, C] per-class reciprocal applied on the way
                out (deferred softmax normalization)."""
                s_psA = ps.tile([128, BC], f32, tag="pA")
                s_psB = ps.tile([32, BC], f32, tag="pB")
                for blk in range(BLK):
                    nc.tensor.matmul(s_psA[:], lhs4[:, blk, 0:8, :],
                                     xt3[:, blk, :],
                                     start=(blk == 0), stop=(blk == BLK - 1))
                for blk in range(BLK):
                    nc.tensor.matmul(s_psB[:], lhs4[:, blk, 8:10, :],
                                     xt3[:, blk, :],
                                     start=(blk == 0), stop=(blk == BLK - 1))
                sA = work.tile([128, BC], f32, tag="sA")
                sB = work.tile([32, BC], f32, tag="sB")
                nc.scalar.mul(sA[:], s_psA[:], scale)
                nc.scalar.mul(sB[:], s_psB[:], scale)
                tTA = ps.tile([BC, 128], f32, tag="pC")
                tTB = ps.tile([BC, 32], f32, tag="pD")
                nc.tensor.transpose(tTA[:], sA[:], ident[:])
                nc.tensor.transpose(tTB[:], sB[:], ident[:32, :32])
                if norm is None:
                    nc.vector.tensor_copy(sT[:, 0:128], tTA[:])
                    nc.vector.tensor_copy(sT[:, 128:160], tTB[:])
                else:
                    nc.vector.tensor_tensor(
                        sT[:, 0:128].rearrange("p (c o) -> p c o", c=8),
                        tTA[:].rearrange("p (c o) -> p c o", c=8),
                        norm[:, 0:8].unsqueeze(2).broadcast_to([BC, 8, O]),
                        Alu.mult)
                    nc.vector.tensor_tensor(
                        sT[:, 128:160].rearrange("p (c o) -> p c o", c=2),
                        tTB[:].rearrange("p (c o) -> p c o", c=2),
                        norm[:, 8:10].unsqueeze(2).broadcast_to([BC, 2, O]),
                        Alu.mult)

            def squash():
                """sT -> (s_mut^T in sT, v^T in vT, new_f in nf)."""
                fv = sT[:, 0:CO:16]                       # [BC, 10] channel-0
                lt3 = lt[:].rearrange("p (a b) -> p a b", a=C)
                nc.vector.tensor_tensor(
                    lt3, fv.unsqueeze(1).broadcast_to([BC, C, C]),
                    fv.unsqueeze(2).broadcast_to([BC, C, C]), Alu.is_lt)
                nc.vector.reduce_sum(rank[:], lt3, axis=mybir.AxisListType.X)
                # counts: i_k = #(f < T_k); then i_k - 1; then gates
                i1, i2, i3 = cnt[:, 0:1], cnt[:, 1:2], cnt[:, 2:3]
                i1m, i2m, i3m = cnt[:, 3:4], cnt[:, 4:5], cnt[:, 5:6]
                g1, g2, g3, g4 = (cnt[:, 6:7], cnt[:, 7:8],
                                  cnt[:, 8:9], cnt[:, 9:10])
                tmp = cnt[:, 10:11]
                for thr, acc in ((T1, i1), (T2, i2), (T3, i3)):
                    nc.vector.tensor_single_scalar(
                        lt[:, 0:C], fv, float(thr), Alu.is_lt)
                    nc.vector.reduce_sum(acc, lt[:, 0:C].unsqueeze(1),
                                         axis=mybir.AxisListType.X)
                nc.vector.tensor_scalar_add(i1m, i1, -1.0)
                nc.vector.tensor_scalar_add(i2m, i2, -1.0)
                nc.vector.tensor_scalar_add(i3m, i3, -1.0)
                nc.vector.tensor_single_scalar(g1, i1, 0.0, Alu.is_gt)
                nc.vector.tensor_single_scalar(tmp, i2, 0.0, Alu.is_gt)
                nc.vector.scalar_tensor_tensor(g2, i2, i1, tmp,
                                               Alu.is_gt, Alu.mult)
                nc.vector.tensor_single_scalar(tmp, i3, 0.0, Alu.is_gt)
                nc.vector.scalar_tensor_tensor(g3, i3, i2, tmp,
                                               Alu.is_gt, Alu.mult)
                nc.vector.tensor_single_scalar(g4, i3, float(C), Alu.is_lt)
                # segments seg_k = a_k * f + b_k
                for k, (a, b) in enumerate(SEGS):
                    nc.vector.tensor_scalar(segs[:, k * C:(k + 1) * C], fv,
                                            float(a), float(b),
                                            Alu.mult, Alu.add)
                # masks (disjoint)
                m1, m2 = msk[:, 0:C], msk[:, C:2 * C]
                m3, m4 = msk[:, 2 * C:3 * C], msk[:, 3 * C:4 * C]
                nc.vector.scalar_tensor_tensor(
                    m1, rank[:], i1m, g1.broadcast_to([BC, C]),
                    Alu.is_lt, Alu.mult)
                nc.vector.scalar_tensor_tensor(
                    m2, rank[:], i1, g2.broadcast_to([BC, C]),
                    Alu.is_ge, Alu.mult)
                nc.vector.scalar_tensor_tensor(
                    m2, rank[:], i2m, m2, Alu.is_lt, Alu.mult)
                nc.vector.scalar_tensor_tensor(
                    m3, rank[:], i2, g3.broadcast_to([BC, C]),
                    Alu.is_ge, Alu.mult)
                nc.vector.scalar_tensor_tensor(
                    m3, rank[:], i3m, m3, Alu.is_lt, Alu.mult)
                nc.vector.scalar_tensor_tensor(
                    m4, rank[:], i3, g4.broadcast_to([BC, C]),
                    Alu.is_ge, Alu.mult)
                nc.vector.scalar_tensor_tensor(
                    m4, rank[:], float(C - 1), m4, Alu.is_lt, Alu.mult)
                # new_f = select chain (masks disjoint, any order)
                nc.vector.tensor_copy(mski[:], msk[:])
                nc.vector.tensor_copy(nf[:], fv)
                for k in range(4):
                    nc.vector.copy_predicated(nf[:], mski[:, k * C:(k + 1) * C],
                                              segs[:, k * C:(k + 1) * C])
                # s_mut channel 0 <- new_f ; v = new_f * s_mut
                nc.vector.tensor_copy(fv, nf[:])
                nfb = nf[:].unsqueeze(2).broadcast_to([BC, C, O])
                nc.vector.tensor_tensor(
                    vT[:].rearrange("p (c o) -> p c o", c=C), sT[:].rearrange(
                        "p (c o) -> p c o", c=C), nfb, Alu.mult)
                nc.vector.tensor_copy(vTd[0:BC, 0:CO], vT[:])
                nc.vector.tensor_copy(vTd[32:32 + BC, 256:256 + CO], vT[:])

            def a_pass(cc_in, cc_out):
                """partial_a -> AllReduce -> ar_sb [16, (blk c)]."""
                GRP = 4
                for g0 in range(0, BLK, GRP):
                    g = g0 // 2
                    g_ps = ps.tile([128, 1024], f32,
                                   tag=("pA" if (g0 // GRP) % 2 == 0 else "pB"))
                    nc.tensor.matmul(
                        g_ps[:, 0:512], xb2_r[:, g * 128:(g + 1) * 128],
                        vTd[:], start=True, stop=True)
                    nc.tensor.matmul(
                        g_ps[:, 512:1024], xb2_r[:, (g + 1) * 128:(g + 2) * 128],
                        vTd[:], start=True, stop=True)
                    prod = work.tile([128, GRP * CO], f32, tag="prod")
                    g_view = g_ps[:].rearrange("p (g n) -> p g n", g=GRP)[
                        :, :, 0:CO]
                    w_view = w_sb[:, g0 * CO:(g0 + GRP) * CO].rearrange(
                        "p (g n) -> p g n", g=GRP)
                    if (g0 // GRP) % 3:
                        nc.vector.tensor_tensor(
                            prod[:].rearrange("p (g n) -> p g n", g=GRP),
                            w_view, g_view, Alu.mult)
                        nc.vector.reduce_sum(
                            t2[:, g0 * C:(g0 + GRP) * C].rearrange(
                                "p (g c) -> p g c", g=GRP),
                            prod[:].rearrange("p (g c o) -> p g c o",
                                              g=GRP, c=C),
                            axis=mybir.AxisListType.X)
                    else:
                        g_sb = work.tile([128, GRP * CO], f32, tag="g_sb")
                        nc.scalar.copy(
                            g_sb[:].rearrange("p (g n) -> p g n", g=GRP),
                            g_view)
                        nc.gpsimd.tensor_tensor(
                            prod[:], w_sb[:, g0 * CO:(g0 + GRP) * CO],
                            g_sb[:], Alu.mult)
                        p4 = prod[:].rearrange("p (g c o) -> p g c o",
                                               g=GRP, c=C)
                        nc.gpsimd.tensor_add(p4[:, :, :, 0:8], p4[:, :, :, 0:8],
                                             p4[:, :, :, 8:16])
                        nc.gpsimd.tensor_add(p4[:, :, :, 0:4], p4[:, :, :, 0:4],
                                             p4[:, :, :, 4:8])
                        nc.gpsimd.tensor_add(p4[:, :, :, 0:2], p4[:, :, :, 0:2],
                                             p4[:, :, :, 2:4])
                        nc.gpsimd.tensor_add(
                            t2[:, g0 * C:(g0 + GRP) * C].rearrange(
                                "p (g c) -> p g c", g=GRP).unsqueeze(3),
                            p4[:, :, :, 0:1], p4[:, :, :, 1:2])
                a16 = ps.tile([16, BLK * C], f32, tag="pC")
                nc.tensor.matmul(a16[:, 0:512], eye16[:], t2[:, 0:512],
                                 start=True, stop=True)
                nc.tensor.matmul(a16[:, 512:BLK * C], eye16[:],
                                 t2[:, 512:BLK * C], start=True, stop=True)
                a16_sb = work.tile([16, BLK * C], f32, tag="a16_sb")
                nc.scalar.copy(a16_sb[:], a16[:])
                nc.sync.dma_start(cc_in[:], a16_sb[:])
                nc.gpsimd.collective_compute(
                    "AllReduce", Alu.add,
                    ins=[cc_in.opt()], outs=[cc_out.opt()],
                    replica_groups=[list(range(NCORES))])
                nc.sync.dma_start(ar_sb[:], cc_out[:])

            def softmax_and_wc(emit_cij=False):
                """wc <- W * exp(b) (unnormalized); rcp13 <- 1/Z per class,
                computed in parallel with the wc multiplies. cij16 is only
                materialized when emit_cij (final iteration, for c_out)."""
                nc.scalar.activation(exp16[:], b16[:], Act.Exp)
                c_ps = ps.tile([128, BLK * C], f32, tag="pC")
                nc.tensor.matmul(c_ps[:, 0:512], e16x128[:], exp16[:, 0:512],
                                 start=True, stop=True)
                nc.tensor.matmul(c_ps[:, 512:BLK * C], e16x128[:],
                                 exp16[:, 512:BLK * C], start=True, stop=True)
                nc.scalar.copy(c_sb[:], c_ps[:])
                cp3 = c_ps[:].rearrange("p (blk c) -> p blk c", blk=BLK)
                cs3 = c_sb[:].rearrange("p (blk c) -> p blk c", blk=BLK)
                for k, ch in enumerate(range(0, BLK, 6)):
                    if k % 3 == 2:
                        nc.gpsimd.tensor_tensor(
                            wc4[:, ch:ch + 6], w4[:, ch:ch + 6],
                            cs3[:, ch:ch + 6].unsqueeze(3)
                            .broadcast_to([128, 6, C, O]), Alu.mult)
                    else:
                        nc.vector.tensor_tensor(
                            wc4[:, ch:ch + 6], w4[:, ch:ch + 6],
                            cp3[:, ch:ch + 6].unsqueeze(3)
                            .broadcast_to([128, 6, C, O]), Alu.mult)
                # normalization (off the wc critical path)
                zr16 = res.tile([16, C], f32, tag="zr16")
                nc.vector.reduce_sum(
                    zr16[:],
                    exp16[:].rearrange("p (blk c) -> p c blk", blk=BLK),
                    axis=mybir.AxisListType.X)
                z_ps = ps.tile([1, C], f32, tag="pD")
                nc.tensor.matmul(z_ps[:], ones16[:], zr16[:],
                                 start=True, stop=True)
                nc.vector.reciprocal(rz[:], z_ps[:])
                r13_ps = ps.tile([BC, C], f32, tag="pE")
                nc.tensor.matmul(r13_ps[:], ones1x16[:, 0:BC], rz[:],
                                 start=True, stop=True)
                nc.vector.tensor_copy(rcp13[:], r13_ps[:])
                if emit_cij:
                    r_ps = ps.tile([16, C], f32, tag="pE")
                    nc.tensor.matmul(r_ps[:], ones1x16[:], rz[:],
                                     start=True, stop=True)
                    nc.vector.tensor_copy(rcp16[:], r_ps[:])
                    nc.vector.tensor_tensor(
                        cij16[:].rearrange("p (blk c) -> p blk c", blk=BLK),
                        exp16[:].rearrange("p (blk c) -> p blk c", blk=BLK),
                        rcp16[:].unsqueeze(1).broadcast_to([16, BLK, C]),
                        Alu.mult)

            cc_in0 = dram.tile([16, BLK * C], f32, tag="cc_in0")
            cc_out0 = dram.tile([16, BLK * C], f32, tag="cc_out0")
            cc_in1 = dram.tile([16, BLK * C], f32, tag="cc_in1")
            cc_out1 = dram.tile([16, BLK * C], f32, tag="cc_out1")

            # ---- iteration 0: c uniform = 1/R ----
            s_pass(w4, 1.0 / R)
            squash()
            a_pass(cc_in0, cc_out0)
            # b16 = ar/100
            nc.vector.tensor_scalar_mul(b16[:], ar_sb[:], 1.0 / B)

            # ---- iteration 1 ----
            softmax_and_wc()
            s_pass(wc4, 1.0, norm=rcp13)
            squash()
            a_pass(cc_in1, cc_out1)
            nc.vector.scalar_tensor_tensor(b16[:], ar_sb[:], 1.0 / B, b16[:],
                                           Alu.mult, Alu.add)

            # ---- iteration 2 (final) ----
            softmax_and_wc(emit_cij=True)
            nc.sync.dma_start(c_out[:], cij16[:])
            s_pass(wc4, 1.0, norm=rcp13)
            squash()
            nc.sync.dma_start(v_out[:], vT[:])
            nc.sync.dma_start(smut_out[:], sT[:])

    nc.finalize()
    return nc


def _make_runner():
    """Build nc once and a cached jitted SPMD executor (axon/PJRT path)."""
    import jax
    import jax.numpy as jnp
    from jax.sharding import Mesh, PartitionSpec
    from jax.experimental.shard_map import shard_map
    import concourse.mybir as mybir
    from concourse import bass2jax

    nc = _build_nc()
    bass2jax.install_neuronx_cc_hook()

    in_names, out_names, out_avals, zero_outs = [], [], [], []
    partition_name = (nc.partition_id_tensor.name
                      if nc.partition_id_tensor else None)
    for alloc in nc.m.functions[0].allocations:
        if not isinstance(alloc, mybir.MemoryLocationSet):
            continue
        name = alloc.memorylocations[0].name
        if alloc.kind == "ExternalInput":
            if name != partition_name:
                in_names.append(name)
        elif alloc.kind == "ExternalOutput":
            shape = tuple(alloc.tensor_shape)
            dtype = mybir.dt.np(alloc.dtype)
            out_names.append(name)
            out_avals.append(jax.core.ShapedArray(shape, dtype))
            zero_outs.append(np.zeros(shape, dtype))
    n_params = len(in_names)
    n_outs = len(out_avals)
    all_in_names = list(in_names) + list(out_names)
    if partition_name is not None:
        all_in_names.append(partition_name)
    donate = tuple(range(n_params, n_params + n_outs))

    def _body(*args):
        operands = list(args)
        if partition_name is not None:
            operands.append(bass2jax.partition_id_tensor())
        outs = bass2jax._bass_exec_p.bind(
            *operands,
            out_avals=tuple(out_avals),
            in_names=tuple(all_in_names),
            out_names=tuple(out_names),
            lowering_input_output_aliases=(),
            sim_require_finite=False,
            sim_require_nnan=False,
            nc=nc,
        )
        return tuple(outs)

    devices = jax.devices()[:NCORES]
    mesh = Mesh(np.asarray(devices), ("core",))
    in_specs = (PartitionSpec("core"),) * (n_params + n_outs)
    out_specs = (PartitionSpec("core"),) * n_outs
    sharded = jax.jit(
        shard_map(_body, mesh=mesh, in_specs=in_specs, out_specs=out_specs,
                  check_rep=False),
        donate_argnums=donate, keep_unused=True)
    _NC[0] = nc
    _IO[0] = (in_names, out_names, out_avals, zero_outs)
    _SHARDED[0] = sharded

    def run(in_maps):
        per_core = [[np.asarray(m[k]) for k in in_names] for m in in_maps]
        concat_in = [np.concatenate([per_core[c][i] for c in range(NCORES)],
                                    axis=0) for i in range(n_params)]
        concat_zeros = [np.zeros((NCORES * z.shape[0], *z.shape[1:]), z.dtype)
                        for z in zero_outs]
        out_arrs = sharded(*concat_in, *concat_zeros)
        outs = [np.asarray(o) for o in out_arrs]
        return [
            {name: outs[i].reshape(NCORES, *out_avals[i].shape)[c]
             for i, name in enumerate(out_names)}
            for c in range(NCORES)
        ]

    return run


def _prep_inputs(x0):
    """Host-side shard/reformat (slicing, transposes, padding only)."""
    x0p = np.zeros((NCORES * BC, R, II), np.float32)
    x0p[:B] = x0
    w = None  # filled by caller
    eye16 = np.zeros((128, 16), np.float32)
    eye16[np.arange(128), np.arange(128) // 8] = 1.0
    e16x128 = np.zeros((16, 128), np.float32)
    e16x128[np.arange(128) // 8, np.arange(128)] = 1.0
    ones16 = np.ones((16, 1), np.float32)
    ones1x16 = np.ones((1, 16), np.float32)
    ident = np.eye(128, dtype=np.float32)
    per_core = []
    for ci in range(NCORES):
        xc = x0p[ci * BC:(ci + 1) * BC]           # [13, 1152, 8]
        x_t = np.ascontiguousarray(
            xc.reshape(BC, BLK, 16, II).transpose(2, 3, 1, 0)
        ).reshape(128, BLK * BC)
        xpair = xc.reshape(BC, BLK // 2, 2, 128)
        x_b2 = np.zeros((64, (BLK // 2) * 128), np.float32)
        x_b2[0:BC] = np.ascontiguousarray(
            xpair[:, :, 0, :]).reshape(BC, -1)
        x_b2[32:32 + BC] = np.ascontiguousarray(
            xpair[:, :, 1, :]).reshape(BC, -1)
        per_core.append({"x_t": x_t, "x_b2": x_b2, "eye16": eye16,
                         "e16x128": e16x128, "ones16": ones16,
                         "ones1x16": ones1x16, "ident": ident})
    return per_core


def kernel(x0, x1, x2, W, train_or_test=0, epch=0):
    x0 = np.asarray(x0, np.float32)
    W = np.asarray(W, np.float32)
    if _RUNNER[0] is None:
        _RUNNER[0] = _make_runner()
    run = _RUNNER[0]

    w_t = np.ascontiguousarray(
        W[0].reshape(BLK, 16, C, O, II).transpose(1, 4, 0, 2, 3)
    ).reshape(128, BLK * CO)
    in_maps = _prep_inputs(x0)
    for m in in_maps:
        m["w_t"] = w_t

    results = run(in_maps)

    v = np.concatenate([r["v_out"] for r in results], axis=0)[:B]
    smut = np.concatenate([r["smut_out"] for r in results], axis=0)[:B]
    c16 = results[0]["c_out"].reshape(16, BLK, C)
    c2 = np.ascontiguousarray(c16.transpose(1, 0, 2)).reshape(R, C)

    v_j = v.reshape(B, C, O, 1)
    s_mut = smut.reshape(B, C, O, 1)
    c_full = np.broadcast_to(c2[None, :, :, None, None], (B, R, C, 1, 1))
    Wb = np.broadcast_to(W, (B,) + W.shape[1:])
    squashed_u = np.broadcast_to(x0[:, :, None, :, None], (B, R, C, II, 1))
    return (v_j, c_full, Wb, squashed_u, np.asarray(x1), np.asarray(x2),
            s_mut)


# revision 22
# speedup vs baseline: 1.0414x; 1.0296x over previous
"""Trainium2 Bass kernel for nn_DigitCaps (capsule dynamic routing with
piecewise-linear squash). Self-contained: hardcodes shapes/sharding.

Sharding: data-parallel over batch (100 -> 8 cores x 13, zero-padded to 104).
W is replicated. The per-routing-iteration mean over batch is an AllReduce of
per-core partial sums of a_ij.

Device algorithm per core (batch chunk Bc=13):
  W stays resident in SBUF as w[(rr,i)=128, (blk=72, c=10, o=16)].
  s_j^t      = sum_{r,i} (c^t[r,c] * W[r,c,o,i]) * x0[b,r,i]   (PE, fp32)
  squash     = rank-based piecewise-linear remap (DVE, no actual sort needed)
  partial_a  = sum_{o,i} W * G,  G[(r,i),(c,o)] = sum_b x0[b,r,i] v[b,c,o]
               (PE for G and the i-reduction, DVE for the o-reduction)
  b_ij      += AllReduce(partial_a) / 100
"""

import numpy as np

B, R, C, O, II = 100, 1152, 10, 16, 8
NCORES = 8
BC = 13          # per-core batch (zero-padded to 8*13 = 104)
BLK = R // 16    # 72 blocks of 16 routes
CO = C * O       # 160

T1, T2, T3 = -0.075410217, 0.0, 0.062207676
SEGS = [(-0.074520095, 0.349297946), (-0.534473989, 0.27196494),
        (0.637642944, 0.295330779), (0.169344703, 0.353784456)]

_RUNNER = [None]
_NC = [None]
_IO = [None]
_SHARDED = [None]


def _build_nc():
    import concourse.bacc as bacc
    import concourse.mybir as mybir
    from concourse import tile

    dt = mybir.dt
    f32 = dt.float32
    Alu = mybir.AluOpType
    Act = mybir.ActivationFunctionType

    nc = bacc.Bacc("TRN2", target_bir_lowering=False, debug=False,
                   num_devices=NCORES)

    w_in = nc.dram_tensor("w_t", [128, BLK * CO], f32, kind="ExternalInput")
    xt_in = nc.dram_tensor("x_t", [128, BLK * BC], f32, kind="ExternalInput")
    xb2_in = nc.dram_tensor("x_b2", [64, (BLK // 2) * 128], f32,
                            kind="ExternalInput")
    eye16_in = nc.dram_tensor("eye16", [128, 16], f32, kind="ExternalInput")
    e16x128_in = nc.dram_tensor("e16x128", [16, 128], f32, kind="ExternalInput")
    ones16_in = nc.dram_tensor("ones16", [16, 1], f32, kind="ExternalInput")
    ones1x16_in = nc.dram_tensor("ones1x16", [1, 16], f32, kind="ExternalInput")
    ident_in = nc.dram_tensor("ident", [128, 128], f32, kind="ExternalInput")

    v_out = nc.dram_tensor("v_out", [BC, CO], f32, kind="ExternalOutput")
    smut_out = nc.dram_tensor("smut_out", [BC, CO], f32, kind="ExternalOutput")
    c_out = nc.dram_tensor("c_out", [16, BLK * C], f32, kind="ExternalOutput")

    with tile.TileContext(nc) as tc:
        with tc.tile_pool(name="res", bufs=1) as res, \
             tc.tile_pool(name="work", bufs=2) as work, \
             tc.tile_pool(name="ps", bufs=1, space="PSUM") as ps, \
             tc.tile_pool(name="dram", bufs=1, space="DRAM") as dram:

            # ---- resident tensors ----
            w_sb = res.tile([128, BLK * CO], f32, tag="w_sb")
            wc_sb = res.tile([128, BLK * CO], f32, tag="wc_sb")
            xt_sb = res.tile([128, BLK * BC], f32, tag="xt_sb")
            xb2_sb = res.tile([64, (BLK // 2) * 128], f32, tag="xb2_sb")
            xb2_r = res.tile([64, (BLK // 2) * 128], dt.float32r,
                             tag="xb2_r")
            eye16 = res.tile([128, 16], f32, tag="eye16")
            e16x128 = res.tile([16, 128], f32, tag="e16x128")
            ones16 = res.tile([16, 1], f32, tag="ones16")
            ones1x16 = res.tile([1, 16], f32, tag="ones1x16")
            ident = res.tile([128, 128], f32, tag="ident")
            b16 = res.tile([16, BLK * C], f32, tag="b16")
            t2 = res.tile([128, BLK * C], f32, tag="t2")

            nc.sync.dma_start(xt_sb[:], xt_in[:])
            nc.sync.dma_start(xb2_sb[:], xb2_in[:])
            WCH = BLK * CO // 8
            _dma_engs = [nc.sync, nc.scalar, nc.sync, nc.scalar,
                         nc.sync, nc.scalar, nc.gpsimd, nc.gpsimd]
            for ch in range(8):
                _dma_engs[ch].dma_start(
                    w_sb[:, ch * WCH:(ch + 1) * WCH],
                    w_in[:, ch * WCH:(ch + 1) * WCH])
            nc.sync.dma_start(eye16[:], eye16_in[:])
            nc.sync.dma_start(e16x128[:], e16x128_in[:])
            nc.sync.dma_start(ones16[:], ones16_in[:])
            nc.sync.dma_start(ones1x16[:], ones1x16_in[:])
            nc.sync.dma_start(ident[:], ident_in[:])

            w4 = w_sb[:].rearrange("p (blk c o) -> p blk c o", blk=BLK, c=C)
            wc4 = wc_sb[:].rearrange("p (blk c o) -> p blk c o", blk=BLK, c=C)
            xt3 = xt_sb[:].rearrange("p (blk b) -> p blk b", blk=BLK)

            # squash scratch (shared across iterations via tags)
            sT = res.tile([BC, CO], f32, tag="sT")        # s^T, then s_mut^T
            vT = res.tile([BC, CO], f32, tag="vT")
            vTd = res.tile([64, 512], dt.float32r, tag="vTd")
            nc.gpsimd.tensor_copy(xb2_r[:], xb2_sb[:])
            nc.vector.tensor_single_scalar(vTd[:], xb2_sb[:, 0:512], 0.0,
                                           Alu.mult)
            lt = res.tile([BC, C * C], f32, tag="lt")
            rank = res.tile([BC, C], f32, tag="rank")
            nf = res.tile([BC, C], f32, tag="nf")
            cnt = res.tile([BC, 12], f32, tag="cnt")      # i1,i2,i3,i1m,...,gates
            segs = res.tile([BC, 4 * C], f32, tag="segs")
            msk = res.tile([BC, 4 * C], f32, tag="msk")
            mski = res.tile([BC, 4 * C], dt.int32, tag="mski")

            exp16 = res.tile([16, BLK * C], f32, tag="exp16")
            cij16 = res.tile([16, BLK * C], f32, tag="cij16")
            rz = res.tile([1, C], f32, tag="rz")
            rcp16 = res.tile([16, C], f32, tag="rcp16")
            rcp13 = res.tile([BC, C], f32, tag="rcp13")
            thrc = res.tile([BC, 3 * C], f32, tag="thrc")
            nc.gpsimd.memset(thrc[:, 0:C], T1)
            nc.gpsimd.memset(thrc[:, C:2 * C], T2)
            nc.gpsimd.memset(thrc[:, 2 * C:3 * C], T3)
            ar_sb = res.tile([16, BLK * C], f32, tag="ar_sb")
            c_sb = res.tile([128, BLK * C], f32, tag="c_sb")

            def s_pass(lhs4, scale, norm=None):
                """s^T [BC, CO] <- (sum over blocks of lhsT.T @ x) transposed.
                norm: optional [BC, C] per-class reciprocal applied on the way
                out (deferred softmax normalization)."""
                s_psA = ps.tile([128, BC], f32, tag="pA")
                s_psB = ps.tile([32, BC], f32, tag="pB")
                for blk in range(BLK):
                    nc.tensor.matmul(s_psA[:], lhs4[:, blk, 0:8, :],
                                     xt3[:, blk, :],
                                     start=(blk == 0), stop=(blk == BLK - 1))
                for blk in range(BLK):
                    nc.tensor.matmul(s_psB[:], lhs4[:, blk, 8:10, :],
                                     xt3[:, blk, :],
                                     start=(blk == 0), stop=(blk == BLK - 1))
                sA = work.tile([128, BC], f32, tag="sA")
                sB = work.tile([32, BC], f32, tag="sB")
                nc.scalar.mul(sA[:], s_psA[:], scale)
                nc.scalar.mul(sB[:], s_psB[:], scale)
                tTA = ps.tile([BC, 128], f32, tag="pC")
                tTB = ps.tile([BC, 32], f32, tag="pD")
                nc.tensor.transpose(tTA[:], sA[:], ident[:])
                nc.tensor.transpose(tTB[:], sB[:], ident[:32, :32])
                if norm is None:
                    nc.vector.tensor_copy(sT[:, 0:128], tTA[:])
                    nc.vector.tensor_copy(sT[:, 128:160], tTB[:])
                else:
                    nc.vector.tensor_tensor(
                        sT[:, 0:128].rearrange("p (c o) -> p c o", c=8),
                        tTA[:].rearrange("p (c o) -> p c o", c=8),
                        norm[:, 0:8].unsqueeze(2).broadcast_to([BC, 8, O]),
                        Alu.mult)
                    nc.vector.tensor_tensor(
                        sT[:, 128:160].rearrange("p (c o) -> p c o", c=2),
                        tTB[:].rearrange("p (c o) -> p c o", c=2),
                        norm[:, 8:10].unsqueeze(2).broadcast_to([BC, 2, O]),
                        Alu.mult)

            def squash():
                """sT -> (s_mut^T in sT, v^T in vT, new_f in nf)."""
                fv = sT[:, 0:CO:16]                       # [BC, 10] channel-0
                lt3 = lt[:].rearrange("p (a b) -> p a b", a=C)
                nc.vector.tensor_tensor(
                    lt3, fv.unsqueeze(1).broadcast_to([BC, C, C]),
                    fv.unsqueeze(2).broadcast_to([BC, C, C]), Alu.is_lt)
                nc.vector.reduce_sum(rank[:], lt3, axis=mybir.AxisListType.X)
                # counts: i_k = #(f < T_k); then i_k - 1; then gates
                i1, i2, i3 = cnt[:, 0:1], cnt[:, 1:2], cnt[:, 2:3]
                i1m, i2m, i3m = cnt[:, 3:4], cnt[:, 4:5], cnt[:, 5:6]
                g1, g2, g3, g4 = (cnt[:, 6:7], cnt[:, 7:8],
                                  cnt[:, 8:9], cnt[:, 9:10])
                tmp = cnt[:, 10:11]
                lt30 = lt[:, 0:3 * C].rearrange("p (t c) -> p t c", t=3)
                nc.vector.tensor_tensor(
                    lt30, fv.unsqueeze(1).broadcast_to([BC, 3, C]),
                    thrc[:].rearrange("p (t c) -> p t c", t=3), Alu.is_lt)
                nc.vector.reduce_sum(cnt[:, 0:3].unsqueeze(1), lt30,
                                     axis=mybir.AxisListType.X)
                nc.vector.tensor_scalar_add(cnt[:, 3:6], cnt[:, 0:3], -1.0)
                nc.vector.tensor_single_scalar(g1, i1, 0.0, Alu.is_gt)
                nc.vector.tensor_single_scalar(tmp, i2, 0.0, Alu.is_gt)
                nc.vector.scalar_tensor_tensor(g2, i2, i1, tmp,
                                               Alu.is_gt, Alu.mult)
                nc.vector.tensor_single_scalar(tmp, i3, 0.0, Alu.is_gt)
                nc.vector.scalar_tensor_tensor(g3, i3, i2, tmp,
                                               Alu.is_gt, Alu.mult)
                nc.vector.tensor_single_scalar(g4, i3, float(C), Alu.is_lt)
                # segments seg_k = a_k * f + b_k
                for k, (a, b) in enumerate(SEGS):
                    nc.vector.tensor_scalar(segs[:, k * C:(k + 1) * C], fv,
                                            float(a), float(b),
                                            Alu.mult, Alu.add)
                # masks (disjoint)
                m1, m2 = msk[:, 0:C], msk[:, C:2 * C]
                m3, m4 = msk[:, 2 * C:3 * C], msk[:, 3 * C:4 * C]
                nc.vector.scalar_tensor_tensor(
                    m1, rank[:], i1m, g1.broadcast_to([BC, C]),
                    Alu.is_lt, Alu.mult)
                nc.vector.scalar_tensor_tensor(
                    m2, rank[:], i1, g2.broadcast_to([BC, C]),
                    Alu.is_ge, Alu.mult)
                nc.vector.scalar_tensor_tensor(
                    m2, rank[:], i2m, m2, Alu.is_lt, Alu.mult)
                nc.vector.scalar_tensor_tensor(
                    m3, rank[:], i2, g3.broadcast_to([BC, C]),
                    Alu.is_ge, Alu.mult)
                nc.vector.scalar_tensor_tensor(
                    m3, rank[:], i3m, m3, Alu.is_lt, Alu.mult)
                nc.vector.scalar_tensor_tensor(
                    m4, rank[:], i3, g4.broadcast_to([BC, C]),
                    Alu.is_ge, Alu.mult)
                nc.vector.scalar_tensor_tensor(
                    m4, rank[:], float(C - 1), m4, Alu.is_lt, Alu.mult)
                # new_f = select chain (masks disjoint, any order)
                nc.vector.tensor_copy(mski[:], msk[:])
                nc.vector.tensor_copy(nf[:], fv)
                for k in range(4):
                    nc.vector.copy_predicated(nf[:], mski[:, k * C:(k + 1) * C],
                                              segs[:, k * C:(k + 1) * C])
                # s_mut channel 0 <- new_f ; v = new_f * s_mut
                nc.vector.tensor_copy(fv, nf[:])
                nfb = nf[:].unsqueeze(2).broadcast_to([BC, C, O])
                nc.vector.tensor_tensor(
                    vT[:].rearrange("p (c o) -> p c o", c=C), sT[:].rearrange(
                        "p (c o) -> p c o", c=C), nfb, Alu.mult)
                nc.vector.tensor_copy(vTd[0:BC, 0:CO], vT[:])
                nc.vector.tensor_copy(vTd[32:32 + BC, 256:256 + CO], vT[:])

            def a_pass(cc_in, cc_out):
                """partial_a -> AllReduce -> ar_sb [16, (blk c)]."""
                GRP = 4
                for g0 in range(0, BLK, GRP):
                    g = g0 // 2
                    g_ps = ps.tile([128, 1024], f32,
                                   tag=("pA" if (g0 // GRP) % 2 == 0 else "pB"))
                    nc.tensor.matmul(
                        g_ps[:, 0:512], xb2_r[:, g * 128:(g + 1) * 128],
                        vTd[:], start=True, stop=True)
                    nc.tensor.matmul(
                        g_ps[:, 512:1024], xb2_r[:, (g + 1) * 128:(g + 2) * 128],
                        vTd[:], start=True, stop=True)
                    prod = work.tile([128, GRP * CO], f32, tag="prod")
                    g_view = g_ps[:].rearrange("p (g n) -> p g n", g=GRP)[
                        :, :, 0:CO]
                    w_view = w_sb[:, g0 * CO:(g0 + GRP) * CO].rearrange(
                        "p (g n) -> p g n", g=GRP)
                    if (g0 // GRP) % 8 not in (0, 3, 6):
                        nc.vector.tensor_tensor(
                            prod[:].rearrange("p (g n) -> p g n", g=GRP),
                            w_view, g_view, Alu.mult)
                        nc.vector.reduce_sum(
                            t2[:, g0 * C:(g0 + GRP) * C].rearrange(
                                "p (g c) -> p g c", g=GRP),
                            prod[:].rearrange("p (g c o) -> p g c o",
                                              g=GRP, c=C),
                            axis=mybir.AxisListType.X)
                    else:
                        g_sb = work.tile([128, GRP * CO], f32, tag="g_sb")
                        nc.scalar.copy(
                            g_sb[:].rearrange("p (g n) -> p g n", g=GRP),
                            g_view)
                        nc.gpsimd.tensor_tensor(
                            prod[:], w_sb[:, g0 * CO:(g0 + GRP) * CO],
                            g_sb[:], Alu.mult)
                        p4 = prod[:].rearrange("p (g c o) -> p g c o",
                                               g=GRP, c=C)
                        nc.gpsimd.tensor_add(p4[:, :, :, 0:8], p4[:, :, :, 0:8],
                                             p4[:, :, :, 8:16])
                        nc.gpsimd.tensor_add(p4[:, :, :, 0:4], p4[:, :, :, 0:4],
                                             p4[:, :, :, 4:8])
                        nc.gpsimd.tensor_add(p4[:, :, :, 0:2], p4[:, :, :, 0:2],
                                             p4[:, :, :, 2:4])
                        nc.gpsimd.tensor_add(
                            t2[:, g0 * C:(g0 + GRP) * C].rearrange(
                                "p (g c) -> p g c", g=GRP).unsqueeze(3),
                            p4[:, :, :, 0:1], p4[:, :, :, 1:2])
                a16 = ps.tile([16, BLK * C], f32, tag="pC")
                nc.tensor.matmul(a16[:, 0:512], eye16[:], t2[:, 0:512],
                                 start=True, stop=True)
                nc.tensor.matmul(a16[:, 512:BLK * C], eye16[:],
                                 t2[:, 512:BLK * C], start=True, stop=True)
                a16_sb = work.tile([16, BLK * C], f32, tag="a16_sb")
                nc.scalar.copy(a16_sb[:], a16[:])
                nc.sync.dma_start(cc_in[:], a16_sb[:])
                nc.gpsimd.collective_compute(
                    "AllReduce", Alu.add,
                    ins=[cc_in.opt()], outs=[cc_out.opt()],
                    replica_groups=[list(range(NCORES))])
                nc.sync.dma_start(ar_sb[:], cc_out[:])

            def softmax_and_wc(emit_cij=False):
                """wc <- W * exp(b) (unnormalized); rcp13 <- 1/Z per class,
                computed in parallel with the wc multiplies. cij16 is only
                materialized when emit_cij (final iteration, for c_out)."""
                nc.scalar.activation(exp16[:], b16[:], Act.Exp)
                c_ps = ps.tile([128, BLK * C], f32, tag="pC")
                nc.tensor.matmul(c_ps[:, 0:512], e16x128[:], exp16[:, 0:512],
                                 start=True, stop=True)
                nc.tensor.matmul(c_ps[:, 512:BLK * C], e16x128[:],
                                 exp16[:, 512:BLK * C], start=True, stop=True)
                nc.scalar.copy(c_sb[:], c_ps[:])
                cp3 = c_ps[:].rearrange("p (blk c) -> p blk c", blk=BLK)
                cs3 = c_sb[:].rearrange("p (blk c) -> p blk c", blk=BLK)
                for k, ch in enumerate(range(0, BLK, 6)):
                    if k % 3 == 2:
                        nc.gpsimd.tensor_tensor(
                            wc4[:, ch:ch + 6], w4[:, ch:ch + 6],
                            cs3[:, ch:ch + 6].unsqueeze(3)
                            .broadcast_to([128, 6, C, O]), Alu.mult)
                    else:
                        nc.vector.tensor_tensor(
                            wc4[:, ch:ch + 6], w4[:, ch:ch + 6],
                            cp3[:, ch:ch + 6].unsqueeze(3)
                            .broadcast_to([128, 6, C, O]), Alu.mult)
                # normalization (off the wc critical path)
                zr16 = res.tile([16, C], f32, tag="zr16")
                nc.vector.reduce_sum(
                    zr16[:],
                    exp16[:].rearrange("p (blk c) -> p c blk", blk=BLK),
                    axis=mybir.AxisListType.X)
                z_ps = ps.tile([1, C], f32, tag="pD")
                nc.tensor.matmul(z_ps[:], ones16[:], zr16[:],
                                 start=True, stop=True)
                nc.vector.reciprocal(rz[:], z_ps[:])
                r13_ps = ps.tile([BC, C], f32, tag="pE")
                nc.tensor.matmul(r13_ps[:], ones1x16[:, 0:BC], rz[:],
                                 start=True, stop=True)
                nc.vector.tensor_copy(rcp13[:], r13_ps[:])
                if emit_cij:
                    r_ps = ps.tile([16, C], f32, tag="pE")
                    nc.tensor.matmul(r_ps[:], ones1x16[:], rz[:],
                                     start=True, stop=True)
                    nc.vector.tensor_copy(rcp16[:], r_ps[:])
                    nc.vector.tensor_tensor(
                        cij16[:].rearrange("p (blk c) -> p blk c", blk=BLK),
                        exp16[:].rearrange("p (blk c) -> p blk c", blk=BLK),
                        rcp16[:].unsqueeze(1).broadcast_to([16, BLK, C]),
                        Alu.mult)

            cc_in0 = dram.tile([16, BLK * C], f32, tag="cc_in0")
            cc_out0 = dram.tile([16, BLK * C], f32, tag="cc_out0")
            cc_in1 = dram.tile([16, BLK * C], f32, tag="cc_in1")
            cc_out1 = dram.tile([16, BLK * C], f32, tag="cc_out1")

            # ---- iteration 0: c uniform = 1/R ----
            s_pass(w4, 1.0 / R)
            squash()
            a_pass(cc_in0, cc_out0)
            # b16 = ar/100
            nc.vector.tensor_scalar_mul(b16[:], ar_sb[:], 1.0 / B)

            # ---- iteration 1 ----
            softmax_and_wc()
            s_pass(wc4, 1.0, norm=rcp13)
            squash()
            a_pass(cc_in1, cc_out1)
            nc.vector.scalar_tensor_tensor(b16[:], ar_sb[:], 1.0 / B, b16[:],
                                           Alu.mult, Alu.add)

            # ---- iteration 2 (final) ----
            softmax_and_wc(emit_cij=True)
            nc.sync.dma_start(c_out[:], cij16[:])
            s_pass(wc4, 1.0, norm=rcp13)
            squash()
            nc.sync.dma_start(v_out[:], vT[:])
            nc.sync.dma_start(smut_out[:], sT[:])

    nc.finalize()
    return nc


def _make_runner():
    """Build nc once and a cached jitted SPMD executor (axon/PJRT path)."""
    import jax
    import jax.numpy as jnp
    from jax.sharding import Mesh, PartitionSpec
    from jax.experimental.shard_map import shard_map
    import concourse.mybir as mybir
    from concourse import bass2jax

    nc = _build_nc()
    bass2jax.install_neuronx_cc_hook()

    in_names, out_names, out_avals, zero_outs = [], [], [], []
    partition_name = (nc.partition_id_tensor.name
                      if nc.partition_id_tensor else None)
    for alloc in nc.m.functions[0].allocations:
        if not isinstance(alloc, mybir.MemoryLocationSet):
            continue
        name = alloc.memorylocations[0].name
        if alloc.kind == "ExternalInput":
            if name != partition_name:
                in_names.append(name)
        elif alloc.kind == "ExternalOutput":
            shape = tuple(alloc.tensor_shape)
            dtype = mybir.dt.np(alloc.dtype)
            out_names.append(name)
            out_avals.append(jax.core.ShapedArray(shape, dtype))
            zero_outs.append(np.zeros(shape, dtype))
    n_params = len(in_names)
    n_outs = len(out_avals)
    all_in_names = list(in_names) + list(out_names)
    if partition_name is not None:
        all_in_names.append(partition_name)
    donate = tuple(range(n_params, n_params + n_outs))

    def _body(*args):
        operands = list(args)
        if partition_name is not None:
            operands.append(bass2jax.partition_id_tensor())
        outs = bass2jax._bass_exec_p.bind(
            *operands,
            out_avals=tuple(out_avals),
            in_names=tuple(all_in_names),
            out_names=tuple(out_names),
            lowering_input_output_aliases=(),
            sim_require_finite=False,
            sim_require_nnan=False,
            nc=nc,
        )
        return tuple(outs)

    devices = jax.devices()[:NCORES]
    mesh = Mesh(np.asarray(devices), ("core",))
    in_specs = (PartitionSpec("core"),) * (n_params + n_outs)
    out_specs = (PartitionSpec("core"),) * n_outs
    sharded = jax.jit(
        shard_map(_body, mesh=mesh, in_specs=in_specs, out_specs=out_specs,
                  check_rep=False),
        donate_argnums=donate, keep_unused=True)
    _NC[0] = nc
    _IO[0] = (in_names, out_names, out_avals, zero_outs)
    _SHARDED[0] = sharded

    def run(in_maps):
        per_core = [[np.asarray(m[k]) for k in in_names] for m in in_maps]
        concat_in = [np.concatenate([per_core[c][i] for c in range(NCORES)],
                                    axis=0) for i in range(n_params)]
        concat_zeros = [np.zeros((NCORES * z.shape[0], *z.shape[1:]), z.dtype)
                        for z in zero_outs]
        out_arrs = sharded(*concat_in, *concat_zeros)
        outs = [np.asarray(o) for o in out_arrs]
        return [
            {name: outs[i].reshape(NCORES, *out_avals[i].shape)[c]
             for i, name in enumerate(out_names)}
            for c in range(NCORES)
        ]

    return run


def _prep_inputs(x0):
    """Host-side shard/reformat (slicing, transposes, padding only)."""
    x0p = np.zeros((NCORES * BC, R, II), np.float32)
    x0p[:B] = x0
    w = None  # filled by caller
    eye16 = np.zeros((128, 16), np.float32)
    eye16[np.arange(128), np.arange(128) // 8] = 1.0
    e16x128 = np.zeros((16, 128), np.float32)
    e16x128[np.arange(128) // 8, np.arange(128)] = 1.0
    ones16 = np.ones((16, 1), np.float32)
    ones1x16 = np.ones((1, 16), np.float32)
    ident = np.eye(128, dtype=np.float32)
    per_core = []
    for ci in range(NCORES):
        xc = x0p[ci * BC:(ci + 1) * BC]           # [13, 1152, 8]
        x_t = np.ascontiguousarray(
            xc.reshape(BC, BLK, 16, II).transpose(2, 3, 1, 0)
        ).reshape(128, BLK * BC)
        xpair = xc.reshape(BC, BLK // 2, 2, 128)
        x_b2 = np.zeros((64, (BLK // 2) * 128), np.float32)
        x_b2[0:BC] = np.ascontiguousarray(
            xpair[:, :, 0, :]).reshape(BC, -1)
        x_b2[32:32 + BC] = np.ascontiguousarray(
            xpair[:, :, 1, :]).reshape(BC, -1)
        per_core.append({"x_t": x_t, "x_b2": x_b2, "eye16": eye16,
                         "e16x128": e16x128, "ones16": ones16,
                         "ones1x16": ones1x16, "ident": ident})
    return per_core


def kernel(x0, x1, x2, W, train_or_test=0, epch=0):
    x0 = np.asarray(x0, np.float32)
    W = np.asarray(W, np.float32)
    if _RUNNER[0] is None:
        _RUNNER[0] = _make_runner()
    run = _RUNNER[0]

    w_t = np.ascontiguousarray(
        W[0].reshape(BLK, 16, C, O, II).transpose(1, 4, 0, 2, 3)
    ).reshape(128, BLK * CO)
    in_maps = _prep_inputs(x0)
    for m in in_maps:
        m["w_t"] = w_t

    results = run(in_maps)

    v = np.concatenate([r["v_out"] for r in results], axis=0)[:B]
    smut = np.concatenate([r["smut_out"] for r in results], axis=0)[:B]
    c16 = results[0]["c_out"].reshape(16, BLK, C)
    c2 = np.ascontiguousarray(c16.transpose(1, 0, 2)).reshape(R, C)

    v_j = v.reshape(B, C, O, 1)
    s_mut = smut.reshape(B, C, O, 1)
    c_full = np.broadcast_to(c2[None, :, :, None, None], (B, R, C, 1, 1))
    Wb = np.broadcast_to(W, (B,) + W.shape[1:])
    squashed_u = np.broadcast_to(x0[:, :, None, :, None], (B, R, C, II, 1))
    return (v_j, c_full, Wb, squashed_u, np.asarray(x1), np.asarray(x2),
            s_mut)


# revision 23
# speedup vs baseline: 1.0634x; 1.0212x over previous
"""Trainium2 Bass kernel for nn_DigitCaps (capsule dynamic routing with
piecewise-linear squash). Self-contained: hardcodes shapes/sharding.

Sharding: data-parallel over batch (100 -> 8 cores x 13, zero-padded to 104).
W is replicated. The per-routing-iteration mean over batch is an AllReduce of
per-core partial sums of a_ij.

Device algorithm per core (batch chunk Bc=13):
  W stays resident in SBUF as w[(rr,i)=128, (blk=72, c=10, o=16)].
  s_j^t      = sum_{r,i} (c^t[r,c] * W[r,c,o,i]) * x0[b,r,i]   (PE, fp32)
  squash     = rank-based piecewise-linear remap (DVE, no actual sort needed)
  partial_a  = sum_{o,i} W * G,  G[(r,i),(c,o)] = sum_b x0[b,r,i] v[b,c,o]
               (PE for G and the i-reduction, DVE for the o-reduction)
  b_ij      += AllReduce(partial_a) / 100
"""

import numpy as np

B, R, C, O, II = 100, 1152, 10, 16, 8
NCORES = 8
BC = 13          # per-core batch (zero-padded to 8*13 = 104)
BLK = R // 16    # 72 blocks of 16 routes
CO = C * O       # 160

T1, T2, T3 = -0.075410217, 0.0, 0.062207676
SEGS = [(-0.074520095, 0.349297946), (-0.534473989, 0.27196494),
        (0.637642944, 0.295330779), (0.169344703, 0.353784456)]

_RUNNER = [None]
_NC = [None]
_IO = [None]
_SHARDED = [None]


def _build_nc():
    import concourse.bacc as bacc
    import concourse.mybir as mybir
    from concourse import tile

    dt = mybir.dt
    f32 = dt.float32
    Alu = mybir.AluOpType
    Act = mybir.ActivationFunctionType

    nc = bacc.Bacc("TRN2", target_bir_lowering=False, debug=False,
                   num_devices=NCORES)

    w_in = nc.dram_tensor("w_t", [128, BLK * CO], f32, kind="ExternalInput")
    xt_in = nc.dram_tensor("x_t", [128, BLK * BC], f32, kind="ExternalInput")
    xb2_in = nc.dram_tensor("x_b2", [64, (BLK // 2) * 128], f32,
                            kind="ExternalInput")
    eye16_in = nc.dram_tensor("eye16", [128, 16], f32, kind="ExternalInput")
    e16x128_in = nc.dram_tensor("e16x128", [16, 128], f32, kind="ExternalInput")
    ones16_in = nc.dram_tensor("ones16", [16, 1], f32, kind="ExternalInput")
    ones1x16_in = nc.dram_tensor("ones1x16", [1, 16], f32, kind="ExternalInput")
    ident_in = nc.dram_tensor("ident", [128, 128], f32, kind="ExternalInput")

    v_out = nc.dram_tensor("v_out", [BC, CO], f32, kind="ExternalOutput")
    smut_out = nc.dram_tensor("smut_out", [BC, CO], f32, kind="ExternalOutput")
    c_out = nc.dram_tensor("c_out", [16, BLK * C], f32, kind="ExternalOutput")

    with tile.TileContext(nc) as tc:
        with tc.tile_pool(name="res", bufs=1) as res, \
             tc.tile_pool(name="work", bufs=3) as work, \
             tc.tile_pool(name="ps", bufs=1, space="PSUM") as ps, \
             tc.tile_pool(name="dram", bufs=1, space="DRAM") as dram:

            # ---- resident tensors ----
            w_sb = res.tile([128, BLK * CO], f32, tag="w_sb")
            wc_sb = res.tile([128, BLK * CO], f32, tag="wc_sb")
            xt_sb = res.tile([128, BLK * BC], f32, tag="xt_sb")
            xb2_sb = res.tile([64, (BLK // 2) * 128], f32, tag="xb2_sb")
            xb2_r = res.tile([64, (BLK // 2) * 128], dt.float32r,
                             tag="xb2_r")
            eye16 = res.tile([128, 16], f32, tag="eye16")
            e16x128 = res.tile([16, 128], f32, tag="e16x128")
            ones16 = res.tile([16, 1], f32, tag="ones16")
            ones1x16 = res.tile([1, 16], f32, tag="ones1x16")
            ident = res.tile([128, 128], f32, tag="ident")
            b16 = res.tile([16, BLK * C], f32, tag="b16")
            t2 = res.tile([128, BLK * C], f32, tag="t2")

            nc.sync.dma_start(xt_sb[:], xt_in[:])
            nc.sync.dma_start(xb2_sb[:], xb2_in[:])
            WCH = BLK * CO // 8
            _dma_engs = [nc.sync, nc.scalar, nc.sync, nc.scalar,
                         nc.sync, nc.scalar, nc.gpsimd, nc.gpsimd]
            for ch in range(8):
                _dma_engs[ch].dma_start(
                    w_sb[:, ch * WCH:(ch + 1) * WCH],
                    w_in[:, ch * WCH:(ch + 1) * WCH])
            nc.sync.dma_start(eye16[:], eye16_in[:])
            nc.sync.dma_start(e16x128[:], e16x128_in[:])
            nc.sync.dma_start(ones16[:], ones16_in[:])
            nc.sync.dma_start(ones1x16[:], ones1x16_in[:])
            nc.sync.dma_start(ident[:], ident_in[:])

            w4 = w_sb[:].rearrange("p (blk c o) -> p blk c o", blk=BLK, c=C)
            wc4 = wc_sb[:].rearrange("p (blk c o) -> p blk c o", blk=BLK, c=C)
            xt3 = xt_sb[:].rearrange("p (blk b) -> p blk b", blk=BLK)

            # squash scratch (shared across iterations via tags)
            sT = res.tile([BC, CO], f32, tag="sT")        # s^T, then s_mut^T
            vT = res.tile([BC, CO], f32, tag="vT")
            vTd = res.tile([64, 512], dt.float32r, tag="vTd")
            nc.gpsimd.tensor_copy(xb2_r[:], xb2_sb[:])
            nc.vector.tensor_single_scalar(vTd[:], xb2_sb[:, 0:512], 0.0,
                                           Alu.mult)
            lt = res.tile([BC, C * C], f32, tag="lt")
            rank = res.tile([BC, C], f32, tag="rank")
            nf = res.tile([BC, C], f32, tag="nf")
            cnt = res.tile([BC, 12], f32, tag="cnt")      # i1,i2,i3,i1m,...,gates
            segs = res.tile([BC, 4 * C], f32, tag="segs")
            msk = res.tile([BC, 4 * C], f32, tag="msk")
            mski = res.tile([BC, 4 * C], dt.int32, tag="mski")

            exp16 = res.tile([16, BLK * C], f32, tag="exp16")
            cij16 = res.tile([16, BLK * C], f32, tag="cij16")
            rz = res.tile([1, C], f32, tag="rz")
            rcp16 = res.tile([16, C], f32, tag="rcp16")
            rcp13 = res.tile([BC, C], f32, tag="rcp13")
            thrc = res.tile([BC, 3 * C], f32, tag="thrc")
            nc.gpsimd.memset(thrc[:, 0:C], T1)
            nc.gpsimd.memset(thrc[:, C:2 * C], T2)
            nc.gpsimd.memset(thrc[:, 2 * C:3 * C], T3)
            ar_sb = res.tile([16, BLK * C], f32, tag="ar_sb")
            c_sb = res.tile([128, BLK * C], f32, tag="c_sb")

            def s_pass(lhs4, scale, norm=None):
                """s^T [BC, CO] <- (sum over blocks of lhsT.T @ x) transposed.
                norm: optional [BC, C] per-class reciprocal applied on the way
                out (deferred softmax normalization)."""
                s_psA = ps.tile([128, BC], f32, tag="pA")
                s_psB = ps.tile([32, BC], f32, tag="pB")
                for blk in range(BLK):
                    nc.tensor.matmul(s_psA[:], lhs4[:, blk, 0:8, :],
                                     xt3[:, blk, :],
                                     start=(blk == 0), stop=(blk == BLK - 1))
                for blk in range(BLK):
                    nc.tensor.matmul(s_psB[:], lhs4[:, blk, 8:10, :],
                                     xt3[:, blk, :],
                                     start=(blk == 0), stop=(blk == BLK - 1))
                sA = work.tile([128, BC], f32, tag="sA")
                sB = work.tile([32, BC], f32, tag="sB")
                nc.scalar.mul(sA[:], s_psA[:], scale)
                nc.scalar.mul(sB[:], s_psB[:], scale)
                tTA = ps.tile([BC, 128], f32, tag="pC")
                tTB = ps.tile([BC, 32], f32, tag="pD")
                nc.tensor.transpose(tTA[:], sA[:], ident[:])
                nc.tensor.transpose(tTB[:], sB[:], ident[:32, :32])
                if norm is None:
                    nc.vector.tensor_copy(sT[:, 0:128], tTA[:])
                    nc.vector.tensor_copy(sT[:, 128:160], tTB[:])
                else:
                    nc.vector.tensor_tensor(
                        sT[:, 0:128].rearrange("p (c o) -> p c o", c=8),
                        tTA[:].rearrange("p (c o) -> p c o", c=8),
                        norm[:, 0:8].unsqueeze(2).broadcast_to([BC, 8, O]),
                        Alu.mult)
                    nc.vector.tensor_tensor(
                        sT[:, 128:160].rearrange("p (c o) -> p c o", c=2),
                        tTB[:].rearrange("p (c o) -> p c o", c=2),
                        norm[:, 8:10].unsqueeze(2).broadcast_to([BC, 2, O]),
                        Alu.mult)

            def squash():
                """sT -> (s_mut^T in sT, v^T in vT, new_f in nf)."""
                fv = sT[:, 0:CO:16]                       # [BC, 10] channel-0
                lt3 = lt[:].rearrange("p (a b) -> p a b", a=C)
                nc.vector.tensor_tensor(
                    lt3, fv.unsqueeze(1).broadcast_to([BC, C, C]),
                    fv.unsqueeze(2).broadcast_to([BC, C, C]), Alu.is_lt)
                nc.vector.reduce_sum(rank[:], lt3, axis=mybir.AxisListType.X)
                # counts: i_k = #(f < T_k); then i_k - 1; then gates
                i1, i2, i3 = cnt[:, 0:1], cnt[:, 1:2], cnt[:, 2:3]
                i1m, i2m, i3m = cnt[:, 3:4], cnt[:, 4:5], cnt[:, 5:6]
                g1, g2, g3, g4 = (cnt[:, 6:7], cnt[:, 7:8],
                                  cnt[:, 8:9], cnt[:, 9:10])
                tmp = cnt[:, 10:11]
                lt30 = lt[:, 0:3 * C].rearrange("p (t c) -> p t c", t=3)
                nc.vector.tensor_tensor(
                    lt30, fv.unsqueeze(1).broadcast_to([BC, 3, C]),
                    thrc[:].rearrange("p (t c) -> p t c", t=3), Alu.is_lt)
                nc.vector.reduce_sum(cnt[:, 0:3].unsqueeze(1), lt30,
                                     axis=mybir.AxisListType.X)
                nc.vector.tensor_scalar_add(cnt[:, 3:6], cnt[:, 0:3], -1.0)
                nc.vector.tensor_single_scalar(g1, i1, 0.0, Alu.is_gt)
                nc.vector.tensor_single_scalar(tmp, i2, 0.0, Alu.is_gt)
                nc.vector.scalar_tensor_tensor(g2, i2, i1, tmp,
                                               Alu.is_gt, Alu.mult)
                nc.vector.tensor_single_scalar(tmp, i3, 0.0, Alu.is_gt)
                nc.vector.scalar_tensor_tensor(g3, i3, i2, tmp,
                                               Alu.is_gt, Alu.mult)
                nc.vector.tensor_single_scalar(g4, i3, float(C), Alu.is_lt)
                # segments seg_k = a_k * f + b_k
                for k, (a, b) in enumerate(SEGS):
                    nc.vector.tensor_scalar(segs[:, k * C:(k + 1) * C], fv,
                                            float(a), float(b),
                                            Alu.mult, Alu.add)
                # masks (disjoint)
                m1, m2 = msk[:, 0:C], msk[:, C:2 * C]
                m3, m4 = msk[:, 2 * C:3 * C], msk[:, 3 * C:4 * C]
                nc.vector.scalar_tensor_tensor(
                    m1, rank[:], i1m, g1.broadcast_to([BC, C]),
                    Alu.is_lt, Alu.mult)
                nc.vector.scalar_tensor_tensor(
                    m2, rank[:], i1, g2.broadcast_to([BC, C]),
                    Alu.is_ge, Alu.mult)
                nc.vector.scalar_tensor_tensor(
                    m2, rank[:], i2m, m2, Alu.is_lt, Alu.mult)
                nc.vector.scalar_tensor_tensor(
                    m3, rank[:], i2, g3.broadcast_to([BC, C]),
                    Alu.is_ge, Alu.mult)
                nc.vector.scalar_tensor_tensor(
                    m3, rank[:], i3m, m3, Alu.is_lt, Alu.mult)
                nc.vector.scalar_tensor_tensor(
                    m4, rank[:], i3, g4.broadcast_to([BC, C]),
                    Alu.is_ge, Alu.mult)
                nc.vector.scalar_tensor_tensor(
                    m4, rank[:], float(C - 1), m4, Alu.is_lt, Alu.mult)
                # new_f = select chain (masks disjoint, any order)
                nc.vector.tensor_copy(mski[:], msk[:])
                nc.vector.tensor_copy(nf[:], fv)
                for k in range(4):
                    nc.vector.copy_predicated(nf[:], mski[:, k * C:(k + 1) * C],
                                              segs[:, k * C:(k + 1) * C])
                # s_mut channel 0 <- new_f ; v = new_f * s_mut
                nc.vector.tensor_copy(fv, nf[:])
                nfb = nf[:].unsqueeze(2).broadcast_to([BC, C, O])
                nc.vector.tensor_tensor(
                    vT[:].rearrange("p (c o) -> p c o", c=C), sT[:].rearrange(
                        "p (c o) -> p c o", c=C), nfb, Alu.mult)
                nc.vector.tensor_copy(vTd[0:BC, 0:CO], vT[:])
                nc.vector.tensor_copy(vTd[32:32 + BC, 256:256 + CO], vT[:])

            def a_pass(cc_in, cc_out):
                """partial_a -> AllReduce -> ar_sb [16, (blk c)]."""
                GRP = 4
                for g0 in range(0, BLK, GRP):
                    g = g0 // 2
                    g_ps = ps.tile([128, 1024], f32,
                                   tag=("pA" if (g0 // GRP) % 2 == 0 else "pB"))
                    nc.tensor.matmul(
                        g_ps[:, 0:512], xb2_r[:, g * 128:(g + 1) * 128],
                        vTd[:], start=True, stop=True)
                    nc.tensor.matmul(
                        g_ps[:, 512:1024], xb2_r[:, (g + 1) * 128:(g + 2) * 128],
                        vTd[:], start=True, stop=True)
                    prod = work.tile([128, GRP * CO], f32, tag="prod")
                    g_view = g_ps[:].rearrange("p (g n) -> p g n", g=GRP)[
                        :, :, 0:CO]
                    w_view = w_sb[:, g0 * CO:(g0 + GRP) * CO].rearrange(
                        "p (g n) -> p g n", g=GRP)
                    if (g0 // GRP) % 8 not in (0, 3, 6):
                        nc.vector.tensor_tensor(
                            prod[:].rearrange("p (g n) -> p g n", g=GRP),
                            w_view, g_view, Alu.mult)
                        nc.vector.reduce_sum(
                            t2[:, g0 * C:(g0 + GRP) * C].rearrange(
                                "p (g c) -> p g c", g=GRP),
                            prod[:].rearrange("p (g c o) -> p g c o",
                                              g=GRP, c=C),
                            axis=mybir.AxisListType.X)
                    else:
                        g_sb = work.tile([128, GRP * CO], f32, tag="g_sb")
                        nc.scalar.copy(
                            g_sb[:].rearrange("p (g n) -> p g n", g=GRP),
                            g_view)
                        nc.gpsimd.tensor_tensor(
                            prod[:], w_sb[:, g0 * CO:(g0 + GRP) * CO],
                            g_sb[:], Alu.mult)
                        p4 = prod[:].rearrange("p (g c o) -> p g c o",
                                               g=GRP, c=C)
                        nc.gpsimd.tensor_add(p4[:, :, :, 0:8], p4[:, :, :, 0:8],
                                             p4[:, :, :, 8:16])
                        nc.gpsimd.tensor_add(p4[:, :, :, 0:4], p4[:, :, :, 0:4],
                                             p4[:, :, :, 4:8])
                        nc.gpsimd.tensor_add(p4[:, :, :, 0:2], p4[:, :, :, 0:2],
                                             p4[:, :, :, 2:4])
                        nc.gpsimd.tensor_add(
                            t2[:, g0 * C:(g0 + GRP) * C].rearrange(
                                "p (g c) -> p g c", g=GRP).unsqueeze(3),
                            p4[:, :, :, 0:1], p4[:, :, :, 1:2])
                a16 = ps.tile([16, BLK * C], f32, tag="pC")
                nc.tensor.matmul(a16[:, 0:512], eye16[:], t2[:, 0:512],
                                 start=True, stop=True)
                nc.tensor.matmul(a16[:, 512:BLK * C], eye16[:],
                                 t2[:, 512:BLK * C], start=True, stop=True)
                a16_sb = work.tile([16, BLK * C], f32, tag="a16_sb")
                nc.scalar.copy(a16_sb[:], a16[:])
                nc.sync.dma_start(cc_in[:], a16_sb[:])
                nc.gpsimd.collective_compute(
                    "AllReduce", Alu.add,
                    ins=[cc_in.opt()], outs=[cc_out.opt()],
                    replica_groups=[list(range(NCORES))])
                nc.sync.dma_start(ar_sb[:], cc_out[:])

            def softmax_and_wc(emit_cij=False):
                """wc <- W * exp(b) (unnormalized); rcp13 <- 1/Z per class,
                computed in parallel with the wc multiplies. cij16 is only
                materialized when emit_cij (final iteration, for c_out)."""
                nc.scalar.activation(exp16[:], b16[:], Act.Exp)
                c_ps = ps.tile([128, BLK * C], f32, tag="pC")
                nc.tensor.matmul(c_ps[:, 0:512], e16x128[:], exp16[:, 0:512],
                                 start=True, stop=True)
                nc.tensor.matmul(c_ps[:, 512:BLK * C], e16x128[:],
                                 exp16[:, 512:BLK * C], start=True, stop=True)
                nc.scalar.copy(c_sb[:], c_ps[:])
                cp3 = c_ps[:].rearrange("p (blk c) -> p blk c", blk=BLK)
                cs3 = c_sb[:].rearrange("p (blk c) -> p blk c", blk=BLK)
                for k, ch in enumerate(range(0, BLK, 6)):
                    if k % 3 == 2:
                        nc.gpsimd.tensor_tensor(
                            wc4[:, ch:ch + 6], w4[:, ch:ch + 6],
                            cs3[:, ch:ch + 6].unsqueeze(3)
                            .broadcast_to([128, 6, C, O]), Alu.mult)
                    else:
                        nc.vector.tensor_tensor(
                            wc4[:, ch:ch + 6], w4[:, ch:ch + 6],
                            cp3[:, ch:ch + 6].unsqueeze(3)
                            .broadcast_to([128, 6, C, O]), Alu.mult)
                # normalization (off the wc critical path)
                zr16 = res.tile([16, C], f32, tag="zr16")
                nc.vector.reduce_sum(
                    zr16[:],
                    exp16[:].rearrange("p (blk c) -> p c blk", blk=BLK),
                    axis=mybir.AxisListType.X)
                z_ps = ps.tile([1, C], f32, tag="pD")
                nc.tensor.matmul(z_ps[:], ones16[:], zr16[:],
                                 start=True, stop=True)
                nc.vector.reciprocal(rz[:], z_ps[:])
                r13_ps = ps.tile([BC, C], f32, tag="pE")
                nc.tensor.matmul(r13_ps[:], ones1x16[:, 0:BC], rz[:],
                                 start=True, stop=True)
                nc.vector.tensor_copy(rcp13[:], r13_ps[:])
                if emit_cij:
                    r_ps = ps.tile([16, C], f32, tag="pE")
                    nc.tensor.matmul(r_ps[:], ones1x16[:], rz[:],
                                     start=True, stop=True)
                    nc.vector.tensor_copy(rcp16[:], r_ps[:])
                    nc.vector.tensor_tensor(
                        cij16[:].rearrange("p (blk c) -> p blk c", blk=BLK),
                        exp16[:].rearrange("p (blk c) -> p blk c", blk=BLK),
                        rcp16[:].unsqueeze(1).broadcast_to([16, BLK, C]),
                        Alu.mult)

            cc_in0 = dram.tile([16, BLK * C], f32, tag="cc_in0")
            cc_out0 = dram.tile([16, BLK * C], f32, tag="cc_out0")
            cc_in1 = dram.tile([16, BLK * C], f32, tag="cc_in1")
            cc_out1 = dram.tile([16, BLK * C], f32, tag="cc_out1")

            # ---- iteration 0: c uniform = 1/R ----
            s_pass(w4, 1.0 / R)
            squash()
            a_pass(cc_in0, cc_out0)
            # b16 = ar/100
            nc.vector.tensor_scalar_mul(b16[:], ar_sb[:], 1.0 / B)

            # ---- iteration 1 ----
            softmax_and_wc()
            s_pass(wc4, 1.0, norm=rcp13)
            squash()
            a_pass(cc_in1, cc_out1)
            nc.vector.scalar_tensor_tensor(b16[:], ar_sb[:], 1.0 / B, b16[:],
                                           Alu.mult, Alu.add)

            # ---- iteration 2 (final) ----
            softmax_and_wc(emit_cij=True)
            nc.sync.dma_start(c_out[:], cij16[:])
            s_pass(wc4, 1.0, norm=rcp13)
            squash()
            nc.sync.dma_start(v_out[:], vT[:])
            nc.sync.dma_start(smut_out[:], sT[:])

    nc.finalize()
    return nc


def _make_runner():
    """Build nc once and a cached jitted SPMD executor (axon/PJRT path)."""
    import jax
    import jax.numpy as jnp
    from jax.sharding import Mesh, PartitionSpec
    from jax.experimental.shard_map import shard_map
    import concourse.mybir as mybir
    from concourse import bass2jax

    nc = _build_nc()
    bass2jax.install_neuronx_cc_hook()

    in_names, out_names, out_avals, zero_outs = [], [], [], []
    partition_name = (nc.partition_id_tensor.name
                      if nc.partition_id_tensor else None)
    for alloc in nc.m.functions[0].allocations:
        if not isinstance(alloc, mybir.MemoryLocationSet):
            continue
        name = alloc.memorylocations[0].name
        if alloc.kind == "ExternalInput":
            if name != partition_name:
                in_names.append(name)
        elif alloc.kind == "ExternalOutput":
            shape = tuple(alloc.tensor_shape)
            dtype = mybir.dt.np(alloc.dtype)
            out_names.append(name)
            out_avals.append(jax.core.ShapedArray(shape, dtype))
            zero_outs.append(np.zeros(shape, dtype))
    n_params = len(in_names)
    n_outs = len(out_avals)
    all_in_names = list(in_names) + list(out_names)
    if partition_name is not None:
        all_in_names.append(partition_name)
    donate = tuple(range(n_params, n_params + n_outs))

    def _body(*args):
        operands = list(args)
        if partition_name is not None:
            operands.append(bass2jax.partition_id_tensor())
        outs = bass2jax._bass_exec_p.bind(
            *operands,
            out_avals=tuple(out_avals),
            in_names=tuple(all_in_names),
            out_names=tuple(out_names),
            lowering_input_output_aliases=(),
            sim_require_finite=False,
            sim_require_nnan=False,
            nc=nc,
        )
        return tuple(outs)

    devices = jax.devices()[:NCORES]
    mesh = Mesh(np.asarray(devices), ("core",))
    in_specs = (PartitionSpec("core"),) * (n_params + n_outs)
    out_specs = (PartitionSpec("core"),) * n_outs
    sharded = jax.jit(
        shard_map(_body, mesh=mesh, in_specs=in_specs, out_specs=out_specs,
                  check_rep=False),
        donate_argnums=donate, keep_unused=True)
    _NC[0] = nc
    _IO[0] = (in_names, out_names, out_avals, zero_outs)
    _SHARDED[0] = sharded

    def run(in_maps):
        per_core = [[np.asarray(m[k]) for k in in_names] for m in in_maps]
        concat_in = [np.concatenate([per_core[c][i] for c in range(NCORES)],
                                    axis=0) for i in range(n_params)]
        concat_zeros = [np.zeros((NCORES * z.shape[0], *z.shape[1:]), z.dtype)
                        for z in zero_outs]
        out_arrs = sharded(*concat_in, *concat_zeros)
        outs = [np.asarray(o) for o in out_arrs]
        return [
            {name: outs[i].reshape(NCORES, *out_avals[i].shape)[c]
             for i, name in enumerate(out_names)}
            for c in range(NCORES)
        ]

    return run


def _prep_inputs(x0):
    """Host-side shard/reformat (slicing, transposes, padding only)."""
    x0p = np.zeros((NCORES * BC, R, II), np.float32)
    x0p[:B] = x0
    w = None  # filled by caller
    eye16 = np.zeros((128, 16), np.float32)
    eye16[np.arange(128), np.arange(128) // 8] = 1.0
    e16x128 = np.zeros((16, 128), np.float32)
    e16x128[np.arange(128) // 8, np.arange(128)] = 1.0
    ones16 = np.ones((16, 1), np.float32)
    ones1x16 = np.ones((1, 16), np.float32)
    ident = np.eye(128, dtype=np.float32)
    per_core = []
    for ci in range(NCORES):
        xc = x0p[ci * BC:(ci + 1) * BC]           # [13, 1152, 8]
        x_t = np.ascontiguousarray(
            xc.reshape(BC, BLK, 16, II).transpose(2, 3, 1, 0)
        ).reshape(128, BLK * BC)
        xpair = xc.reshape(BC, BLK // 2, 2, 128)
        x_b2 = np.zeros((64, (BLK // 2) * 128), np.float32)
        x_b2[0:BC] = np.ascontiguousarray(
            xpair[:, :, 0, :]).reshape(BC, -1)
        x_b2[32:32 + BC] = np.ascontiguousarray(
            xpair[:, :, 1, :]).reshape(BC, -1)
        per_core.append({"x_t": x_t, "x_b2": x_b2, "eye16": eye16,
                         "e16x128": e16x128, "ones16": ones16,
                         "ones1x16": ones1x16, "ident": ident})
    return per_core


def kernel(x0, x1, x2, W, train_or_test=0, epch=0):
    x0 = np.asarray(x0, np.float32)
    W = np.asarray(W, np.float32)
    if _RUNNER[0] is None:
        _RUNNER[0] = _make_runner()
    run = _RUNNER[0]

    w_t = np.ascontiguousarray(
        W[0].reshape(BLK, 16, C, O, II).transpose(1, 4, 0, 2, 3)
    ).reshape(128, BLK * CO)
    in_maps = _prep_inputs(x0)
    for m in in_maps:
        m["w_t"] = w_t

    results = run(in_maps)

    v = np.concatenate([r["v_out"] for r in results], axis=0)[:B]
    smut = np.concatenate([r["smut_out"] for r in results], axis=0)[:B]
    c16 = results[0]["c_out"].reshape(16, BLK, C)
    c2 = np.ascontiguousarray(c16.transpose(1, 0, 2)).reshape(R, C)

    v_j = v.reshape(B, C, O, 1)
    s_mut = smut.reshape(B, C, O, 1)
    c_full = np.broadcast_to(c2[None, :, :, None, None], (B, R, C, 1, 1))
    Wb = np.broadcast_to(W, (B,) + W.shape[1:])
    squashed_u = np.broadcast_to(x0[:, :, None, :, None], (B, R, C, II, 1))
    return (v_j, c_full, Wb, squashed_u, np.asarray(x1), np.asarray(x2),
            s_mut)
